# revision 22
# baseline (speedup 1.0000x reference)
"""Trainium2 Bass kernel for causal GQA self-attention (S=2048, D=4096, H=32,
HKV=8, DH=128), tensor-parallel over 8 NeuronCores.

Sharding: head-parallel TP. Core i owns q-heads [4i..4i+4) and kv-head i:
  - qkv_proj column shard -> q [S,512], k [S,128], v [S,128]
  - RoPE + causal attention for its 4 heads (GQA group shares the kv head)
  - o_proj row shard (rows [512i..512i+512)) -> bf16 partial [S, D]
Host sums the 8 partials (the "all-reduce") and reshapes to [S, 1, D].

v2 design ("S^T-direct"): attention scores are computed directly in kv-major
layout, sT[kv, q] = K·Q^T, using the dh-major K/Q slabs that the projection
already produces — this removes all 544 PE transposes of P and the 544
PSUM->SBUF copies that made the v1 pipeline DVE-bound and let HAM oscillate.
The softmax denominator l[q] = sum_kv exp(s) is computed on the tensor engine
with an all-ones [128,128] stationary operand: same N-stream cost as any MM,
and the result lands replicated on all 128 partitions, so the per-q reciprocal
can be applied to ctx^T with a single DVE multiply (no partition broadcast).

Per-core phases, interleaved per 512-row sequence chunk n (PE stays dense):
  1. qkv projection, 6 slabs of 128 cols (4q + k + v), N=512 streams;
     q/k slabs RoPE'd in place, v slab PE-transposed to seq-major tiles.
  2. attention for q-chunk c=n: per kv-tile t: sT MM (causal suffix only) ->
     diag mask add (DVE) -> exp (scalar, direct to SBUF bf16) -> l MM + PV MM,
     software-pipelined by 2 tiles so the PE never waits on the scalar exp.
  3. o_proj row shard for chunk c=n, N=512 streams, bf16 partial out.

All matmuls run in bf16 with fp32 PSUM accumulation. Softmax runs without
max-subtraction (logits are O(10) here, far inside fp32 exp range).
"""

import sys

sys.path.insert(0, "/opt/trn_rl_repo")

import numpy as np
import ml_dtypes
from contextlib import ExitStack

import concourse.bass as bass
import concourse.tile as tile
from concourse import mybir
from concourse.bass_utils import run_bass_kernel_spmd
from concourse.masks import make_identity

S, B, D = 2048, 1, 4096
H, HKV, DH = 32, 8, 128
NCORES = 8
HQ = H // HKV  # q heads per core = 4
NSLAB = HQ + 2  # 4 q slabs + k + v
THETA = 10000.0
SCALE = 1.0 / float(np.sqrt(DH))

BF16 = mybir.dt.bfloat16
F32 = mybir.dt.float32
np_bf16 = ml_dtypes.bfloat16

NKB = D // 128  # 32 contraction blocks for the projections
NQB = S // 128  # 16 seq blocks of 128
NCHUNK = S // 512  # 4 sequence chunks of 512


def build_kernel() -> bass.Bass:
    nc = bass.Bass()

    # hidT packed host-side as contiguous [chunk, kb, 128, 512] tiles so each
    # DMA is one dense 128KB block (strided 1KB rows run at ~half DMA rate)
    hidT_e = nc.declare_dram_parameter(
        "hidT", [NCHUNK, NKB // 2, 128, 2, 512], BF16, isOutput=False
    )
    # wqkv split into two contiguous tensors, cols [k|v] and [q0..q3], so the
    # ramp can deliver the k+v columns of a kb block (64KB) independently of
    # its q columns: the first matmul starts ~2.5us earlier and each kb
    # unlocks in 3 sub-deliveries instead of one 320KB step
    wkv_e = nc.declare_dram_parameter("wkv", [D, 2 * DH], BF16, isOutput=False)
    wq_e = nc.declare_dram_parameter("wq", [D, HQ * DH], BF16, isOutput=False)
    wo_e = nc.declare_dram_parameter("wo", [HQ * DH, D], BF16, isOutput=False)
    # cos2 = [cos; cos], sinS = [-sin; sin]  (dh-major halves stacked)
    cos_e = nc.declare_dram_parameter("cos2", [128, S], BF16, isOutput=False)
    sin_e = nc.declare_dram_parameter("sinS", [128, S], BF16, isOutput=False)
    # out packed [qb, dc, 128, 512]; host unpacks to [S, D]
    out_e = nc.declare_dram_parameter("out", [NQB, 8, 128, 512], BF16, isOutput=True)

    hidT = hidT_e[:]
    wkv = wkv_e[:]
    wq = wq_e[:]
    wo = wo_e[:]
    out = out_e[:]

    with tile.TileContext(nc) as tc, ExitStack() as ctx:
        singles = ctx.enter_context(tc.tile_pool(name="singles", bufs=1))

        # ---- persistent SBUF state ----
        wqkv_sb = singles.tile([128, NKB, NSLAB * DH], BF16)
        wo_sb = singles.tile([128, HQ, D], BF16)
        cos_sb = singles.tile([128, S], BF16)
        sin_sb = singles.tile([128, S], BF16)
        ident = singles.tile([128, 128], BF16)
        ones_sb = singles.tile([128, 128], BF16)
        maskT = singles.tile([128, 128], F32)
        # q slabs (m=0..3) + k slab (m=4), dh-major [dh, S], RoPE'd
        qkT_sb = singles.tile([128, HQ + 1, S], BF16)
        # V seq-major: tile t = rows [128t..128t+128) x [dh 128]
        v_sb = singles.tile([128, NQB, DH], BF16)
        # ctx^T per q-head slab [dh, S], softmax-normalized
        ctxT_sb = singles.tile([128, HQ, S], BF16)

        make_identity(nc, ident)
        nc.vector.memset(ones_sb, 1.0)
        # maskT[kv, q] = 0 where q >= kv (valid, diag incl), else -1e9
        nc.gpsimd.memset(maskT, 0.0)
        nc.gpsimd.affine_select(
            out=maskT,
            in_=maskT,
            compare_op=mybir.AluOpType.is_ge,
            fill=-1e9,
            base=0,
            pattern=[[1, 128]],
            channel_multiplier=-1,
        )

        with (
            tc.tile_pool(name="hidp", bufs=22) as hidp,
            tc.tile_pool(name="ropep", bufs=4) as ropep,
            tc.tile_pool(name="vtmp", bufs=2) as vtmpp,
            tc.tile_pool(name="ptp", bufs=4) as ptp,
            tc.tile_pool(name="paccp", bufs=2) as paccp,
            tc.tile_pool(name="linvp", bufs=2) as linvp,
            tc.tile_pool(name="outsb", bufs=4) as osp,
            # PSUM budget: 3 (mm: proj slabs + sT) + 1 (l) + 2 (ctx) + 2
            # (out: o_proj + v-transpose) = 8 banks
            tc.tile_pool(name="ps_mm", bufs=3, space="PSUM") as mmp,
            tc.tile_pool(name="ps_l", bufs=1, space="PSUM") as lpp,
            tc.tile_pool(name="ps_ctx", bufs=2, space="PSUM") as cpp,
            tc.tile_pool(name="ps_out", bufs=2, space="PSUM") as opp,
        ):
            # Deferred per-head softmax normalize: the final l matmul + the
            # Ln/Exp(-x) reciprocal + ctx multiply are emitted under PE cover
            # of the NEXT head's first score matmuls (or the next chunk's
            # first projection matmuls), so the exp->l-accumulate latency
            # never stalls the tensor engine.
            pending = [None]

            def flush_pending():
                if pending[0] is None:
                    return
                cc, hh, l_ps_, ctx_ps_, pacc_ = pending[0]
                pending[0] = None
                if pacc_ is not None:
                    nc.tensor.matmul(l_ps_, ones_sb, pacc_, start=True, stop=True)
                lnl = linvp.tile([128, 512], F32, name=f"lnl_{cc}_{hh}", tag="lnl")
                nc.scalar.activation(lnl, l_ps_, mybir.ActivationFunctionType.Ln)
                linv = linvp.tile(
                    [128, 512], F32, name=f"linv_{cc}_{hh}", tag="linv"
                )
                nc.scalar.activation(
                    linv, lnl, mybir.ActivationFunctionType.Exp, scale=-1.0
                )
                nc.vector.tensor_mul(
                    ctxT_sb[:, hh, cc * 512:(cc + 1) * 512], ctx_ps_, linv
                )

            def issue_ht(n):
                # paired DMAs (2 kb-blocks per transfer): the sync sequencer
                # dispatches each dma_start serially at ~0.6us, so transfer
                # count, not just bytes, gates chunk-0 delivery
                htp = []
                for kp in range(NKB // 2):
                    # first few transfers split into kb singles (interleaved
                    # kv-cols / ht / q-cols so the very first matmul's inputs
                    # land first); afterwards kb-pairs win on dispatch count
                    split = n == 0 and kp < 3
                    t_ = hidp.tile(
                        [128, 2, 512], BF16, name=f"ht_{n}_{kp}", tag="ht"
                    )
                    if n == 0:
                        if split:
                            for j in range(2):
                                kb = 2 * kp + j
                                nc.sync.dma_start(
                                    out=wqkv_sb[:, kb, 0:256],
                                    in_=wkv[kb * 128:(kb + 1) * 128, :],
                                )
                                nc.sync.dma_start(
                                    out=t_[:, j, :], in_=hidT[n, kp, :, j, :]
                                )
                                nc.sync.dma_start(
                                    out=wqkv_sb[:, kb, 256:768],
                                    in_=wq[kb * 128:(kb + 1) * 128, :],
                                )
                        else:
                            nc.sync.dma_start(
                                out=wqkv_sb[:, 2 * kp:2 * kp + 2, 0:256],
                                in_=wkv[kp * 256:(kp + 1) * 256, :].rearrange(
                                    "(j p) c -> p j c", p=128
                                ),
                            )
                            nc.sync.dma_start(
                                out=t_,
                                in_=hidT[n, kp],
                            )
                            nc.sync.dma_start(
                                out=wqkv_sb[:, 2 * kp:2 * kp + 2, 256:768],
                                in_=wq[kp * 256:(kp + 1) * 256, :].rearrange(
                                    "(j p) c -> p j c", p=128
                                ),
                            )
                        if kp == 2:
                            nc.sync.dma_start(out=cos_sb, in_=cos_e[:])
                            nc.sync.dma_start(out=sin_sb, in_=sin_e[:])
                    else:
                        nc.sync.dma_start(
                            out=t_,
                            in_=hidT[n, kp],
                        )
                    htp.append(t_)
                return htp

            htp_map = {0: issue_ht(0)}

            def make_chain(oc, dp_idx, ps_pool, ps_tag, fine_dma=False):
                iq, dp = dp_idx // 4, dp_idx % 4

                def chain():
                    qb = 4 * oc + iq
                    out_sb = osp.tile(
                        [128, 2, 512], BF16, name="out_sb", tag="out_sb"
                    )
                    for half in range(2):
                        dc = 2 * dp + half
                        out_ps = ps_pool.tile(
                            [128, 512], F32, name=f"out_ps_{qb}_{dc}",
                            tag=ps_tag,
                        )
                        for h in range(HQ):
                            nc.tensor.matmul(
                                out_ps,
                                ctxT_sb[:, h, qb * 128:(qb + 1) * 128],
                                wo_sb[:, h, dc * 512:(dc + 1) * 512],
                                start=(h == 0),
                                stop=(h == HQ - 1),
                            )
                        if fine_dma:
                            # kernel tail: drain in quarters on both engines
                            # in parallel so the final DMA fires sooner
                            nc.scalar.copy(out_sb[:, half, 0:256], out_ps[:, 0:256])
                            nc.vector.tensor_copy(
                                out_sb[:, half, 256:512], out_ps[:, 256:512]
                            )
                        elif half == 0:
                            nc.scalar.copy(out_sb[:, 0, :], out_ps)
                        else:
                            nc.vector.tensor_copy(out_sb[:, 1, :], out_ps)
                    if fine_dma:
                        for half in range(2):
                            nc.sync.dma_start(
                                out=out[qb, 2 * dp + half],
                                in_=out_sb[:, half, :],
                            )
                    else:
                        nc.sync.dma_start(
                            out=out[qb, 2 * dp:2 * dp + 2].rearrange(
                                "a p c -> p a c"
                            ),
                            in_=out_sb,
                        )

                return chain

            vts = {}

            def emit_proj(n):
                # ---- qkv projection for seq chunk n ----
                ht = [htp_map[n][kb // 2][:, kb % 2, :] for kb in range(NKB)]
                sl = slice(n * 512, (n + 1) * 512)
                # k slab first so its RoPE is long done when attention starts;
                # v 5th so its staging copy beats the h0 transposes;
                # (slab_idx in qkT_sb/v, column offset in wqkv)
                # All 6 slabs run kb-synchronously: hidp ring slots then
                # free progressively through the phase (slab-sequential order
                # frees them only during the last slab's sweep, compressing
                # the next chunk's hidT delivery into the phase tail), and at
                # chunk 0 the PE work unlocked per arriving (wkv, ht, wq)
                # kb-pair matches the DMA ramp's delivery rate.
                # Accumulator rings, chosen by when their previous occupant
                # frees: k first (ctx ring slot freed in the prior attention),
                # v/q0/q1 on the mm ring (sT slots), q3 on the l ring (freed
                # by the pending flush's early Ln), q2 last on the ctx ring
                # (freed by that flush's final DVE multiply, ~1.6us in).
                # Emission order also matches the ramp delivery order
                # kv-cols -> ht -> q-cols of each kb block.
                # order: the mm-ring slabs (v,q0,q1) finish first (their
                # PSUM slots are reused by the next attention's first sT
                # tiles within ~1us), then k (ctx slot, reused at ~+1.3us),
                # then q3/q2 whose borrowed l/ctx slots aren't touched for
                # 4-8us. The v slab leads so its staging copy is long done
                # before the attention-h0 transposes.
                SLABS = ((5, 128), (0, 256), (1, 384), (4, 0), (3, 640), (2, 512))
                groups = [SLABS]
                slab_pools = [mmp, mmp, mmp, cpp, lpp, cpp]
                slab_tags = ["mm", "mm", "mm", "ctx", "l", "ctx"]
                vt_box = [None]

                def finish_slab(m, ps, on_vector=False):
                    if m < NSLAB - 1:
                        # q or k slab: copy out of PSUM, then RoPE in place.
                        # The last two slabs to finish drain on the vector
                        # engine: their copies fire at phase end, and on the
                        # scalar engine they would queue in front of the next
                        # attention's exps (the attention critical path).
                        slab = qkT_sb[:, m, sl]
                        if on_vector:
                            nc.vector.tensor_copy(slab, ps)
                        else:
                            nc.scalar.copy(slab, ps)
                        rot = ropep.tile([128, 512], BF16, name="rot", tag="rot")
                        nc.sync.dma_start(out=rot[0:64, :], in_=qkT_sb[64:128, m, sl])
                        nc.sync.dma_start(out=rot[64:128, :], in_=qkT_sb[0:64, m, sl])
                        rt = ropep.tile([128, 512], BF16, name="rt", tag="rt")
                        nc.vector.tensor_mul(rt, rot, sin_sb[:, sl])
                        nc.vector.tensor_mul(slab, slab, cos_sb[:, sl])
                        nc.vector.tensor_add(slab, slab, rt)
                    else:
                        # v slab: stage to SBUF; PE-transposed to seq-major
                        # inside the attention A-loop (keeps PE dense)
                        vt_box[0] = vtmpp.tile([128, 512], BF16, name="vt", tag="vt")
                        nc.scalar.copy(vt_box[0], ps)

                grp = groups[0]
                pss = []
                for si, (m, _) in enumerate(grp):
                    pss.append(
                        slab_pools[si].tile(
                            [128, 512], F32, name=f"proj_ps_{n}_{m}",
                            tag=slab_tags[si],
                        )
                    )
                # last head's softmax normalize of the chunk-before-last.
                # Must precede the kb loop: q2's first matmul waits on the
                # ctx-ring slot this flush's DVE multiply frees, and that
                # multiply's l matmul must come first in the PE queue.
                flush_pending()
                # Each slab's sweep lags the previous by 2 kb: the six PSUM
                # drains then fire ~2.5us apart, each overlapped by the later
                # slabs' remaining matmuls, instead of bunching at phase end
                # in front of the next attention's exps in the scalar queue.
                # It also gives the ramp's q-piece deliveries (chunk 0) 4+
                # kb of slack behind their kv/ht pieces, and the borrowed
                # l/ctx-ring slots (q3/q2) time to clear the pending flush.
                LAGS = (0, 2, 4, 6, 8, 10)
                for step in range(NKB + LAGS[-1] + 1):
                    for gi, (m, coff) in enumerate(grp):
                        kb_g = step - LAGS[gi]
                        if not (0 <= kb_g < NKB):
                            continue
                        nc.tensor.matmul(
                            pss[gi],
                            wqkv_sb[:, kb_g, coff:coff + 128],
                            ht[kb_g],
                            start=(kb_g == 0),
                            stop=(kb_g == NKB - 1),
                        )
                        if kb_g == NKB - 1:
                            # this slab is done: drain + RoPE it now, under
                            # cover of the remaining slabs' matmuls
                            finish_slab(m, pss[gi], on_vector=(gi >= 4))
                    if step == NKB - 1 and n + 1 < NCHUNK:
                        # prefetch next chunk's hidden states (most finishes
                        # already emitted; their rot DMAs never head-of-line-
                        # block these dispatches for long)
                        htp_map[n + 1] = issue_ht(n + 1)
                vts[n] = vt_box[0]
                if n == 0:
                    # o_proj weights: first needed by the chains in attn c1
                    for h in range(HQ):
                        nc.sync.dma_start(
                            out=wo_sb[:, h, :], in_=wo[h * 128:(h + 1) * 128, :]
                        )

            def emit_attention(c):
                # ---- attention for q-chunk c, with o_proj chains for chunk
                # c-1 interleaved between score tiles. The attention inner
                # loop is scalar-bound (exp of a [128,512] tile ~530ns vs
                # ~432ns of PE per tile), so without filler the PE idles
                # ~100ns/tile waiting on exp; each interleaved chain adds
                # ~1.7us of exp-independent PE work.
                vt = vts.pop(c)
                chains = []
                if c >= 1:
                    chains = [
                        make_chain(c - 1, k, opp, "out") for k in range(16)
                    ]
                ci = [0]

                def emit_chain():
                    if ci[0] < len(chains):
                        chains[ci[0]]()
                        ci[0] += 1

                ntile = 4 * (c + 1)
                slots = HQ * ntile
                stride = max(1, slots // 16)
                slot = [0]
                for h in range(HQ):
                    # (t, qoff, w): kv tile t covers seq [128t, 128t+128); for
                    # diagonal tiles only q >= 128t attends -> stream suffix
                    tiles = []
                    for t in range(ntile):
                        qoff = max(0, 128 * (t - 4 * c))
                        tiles.append((t, qoff, 512 - qoff))
                    nt = len(tiles)
                    l_ps = lpp.tile([128, 512], F32, name=f"l_ps_{c}_{h}", tag="l")
                    ctx_ps = cpp.tile(
                        [128, 512], F32, name=f"ctx_ps_{c}_{h}", tag="ctx"
                    )
                    # c>=1: sum pT tiles on DVE; a single ones-matmul in the
                    # flush then reduces over kv (one PE stream instead of one
                    # per tile). c=0 keeps the per-tile ones-matmuls: A0 has
                    # no o_proj chains to fill the exp-bound stretches, and
                    # the l matmuls (which run after each exp) are free PE
                    # fill there.
                    if c >= 1:
                        pacc = paccp.tile(
                            [128, 512], BF16, name=f"pacc_{c}_{h}", tag="pacc"
                        )
                        nc.vector.memset(pacc, 0.0)
                    else:
                        pacc = None
                    pT_tiles = {}

                    def stage_a(i):
                        t, qoff, w = tiles[i]
                        sT = mmp.tile(
                            [128, 512], F32, name=f"sT_{c}_{h}_{t}", tag="mm"
                        )
                        nc.tensor.matmul(
                            sT[:, 0:w],
                            qkT_sb[:, HQ, t * 128:(t + 1) * 128],
                            qkT_sb[:, h, c * 512 + qoff:(c + 1) * 512],
                            start=True,
                            stop=True,
                        )
                        if t >= 4 * c:
                            # diagonal tile: first 128 streamed q's need mask
                            nc.vector.tensor_add(sT[:, 0:128], sT[:, 0:128], maskT)
                        pT = ptp.tile([128, 512], BF16, name=f"pT_{c}_{h}_{t}", tag="pt")
                        nc.scalar.activation(
                            pT[:, 0:w],
                            sT[:, 0:w],
                            mybir.ActivationFunctionType.Exp,
                            scale=SCALE,
                        )
                        pT_tiles[i] = pT
                        if pacc is not None:
                            # accumulate sum-over-tiles on DVE; the single
                            # all-ones matmul in the flush reduces over kv
                            nc.vector.tensor_add(
                                pacc[:, qoff:512], pacc[:, qoff:512], pT[:, 0:w]
                            )

                    def stage_c(i):
                        t, qoff, w = tiles[i]
                        pT = pT_tiles.pop(i)
                        first, last = i == 0, i == nt - 1
                        if pacc is None:
                            nc.tensor.matmul(
                                l_ps[:, qoff:512],
                                ones_sb,
                                pT[:, 0:w],
                                start=first,
                                stop=last,
                            )
                        nc.tensor.matmul(
                            ctx_ps[:, qoff:512],
                            v_sb[:, t, :],
                            pT[:, 0:w],
                            start=first,
                            stop=last,
                        )

                    DEPTH = 2
                    vt0 = max(0, 4 * c - 4)
                    for i in range(nt):
                        stage_a(i)
                        if h == 0 and vt0 <= i < vt0 + 4:
                            # chunk c's v tiles: dh-major -> seq-major, first
                            # needed by stage_c(4c); scheduled 4 tiles early
                            # so the DVE copies run mid-head, clear of the
                            # head-start rope/chain burst on DVE
                            j = i - vt0
                            vtp = opp.tile([128, 128], BF16, name="vtp", tag="out")
                            nc.tensor.transpose(
                                vtp, vt[:, j * 128:(j + 1) * 128], ident
                            )
                            nc.vector.tensor_copy(v_sb[:, 4 * c + j, :], vtp)
                        if i == 1:
                            flush_pending()
                        if i >= DEPTH:
                            stage_c(i - DEPTH)
                        slot[0] += 1
                        if slot[0] % stride == 1 or stride == 1:
                            emit_chain()
                    for i in range(max(0, nt - DEPTH), nt):
                        stage_c(i)
                    pending[0] = (c, h, l_ps, ctx_ps, pacc)
                while ci[0] < len(chains):
                    emit_chain()

            # phase order P0 P1 A0 P2 A1 P3 A2 A3 tail: attention c runs
            # after proj c+1, whose matmuls cover chunk c's PSUM drains and
            # RoPE chains (otherwise they serialize at the boundary)
            for n in range(NCHUNK):
                emit_proj(n)
                if n >= 1:
                    emit_attention(n - 1)
            flush_pending()
            emit_attention(NCHUNK - 1)

            # ---- tail: o_proj for the final chunk. The attention mm ring
            # (3 PSUM banks) is idle now; alternating out_ps between it and
            # the out ring gives a 5-deep ring so the PE never waits on the
            # scalar/vector PSUM drains.
            flush_pending()
            for k in range(16):
                pool, tag = ((mmp, "mm"), (opp, "out"))[k % 2]
                make_chain(NCHUNK - 1, k, pool, tag, fine_dma=(k >= 14))()
            flush_pending()

    return nc


def _legalize_waits(j):
    """Split multi-wait instructions: the TPB ISA gives each instruction (and
    each dynamic-DMA descriptor) a single semaphore-wait slot, and this walrus
    build errors on extras instead of splitting them. Hoist all but one wait
    into standalone EventSemaphore instructions on the issuing engine, placed
    immediately before the instruction (engine streams execute in program
    order, so the waits complete before the op issues / the descriptor posts).
    """
    n_new = 0
    for fn in j["functions"]:
        for bb in fn["blocks"]:
            insts = bb.get("instructions", [])
            out = []
            for inst in insts:
                si = inst.get("sync_info") or {}
                waits = si.get("on_wait") or []
                if len(waits) > 1:
                    for w in waits[:-1]:
                        n_new += 1
                        out.append(
                            {
                                "name": f"{inst['name']}-lw{n_new}",
                                "opcode": "EventSemaphore",
                                "engine": inst["engine"],
                                "ins": [],
                                "outs": [],
                                "debug": inst.get("debug"),
                                "sync_info": {"on_update": [], "on_wait": [w]},
                            }
                        )
                    si = dict(si)
                    si["on_wait"] = [waits[-1]]
                    inst = dict(inst)
                    inst["sync_info"] = si
                out.append(inst)
            bb["instructions"] = out
    return j


def _patch_json(nc):
    import json

    orig = nc.to_json_bytes

    def patched():
        j = json.loads(orig())
        return json.dumps(_legalize_waits(j)).encode()

    nc.to_json_bytes = patched
    return nc


_NC_CACHE = None


def _get_nc():
    global _NC_CACHE
    if _NC_CACHE is None:
        _NC_CACHE = _patch_json(build_kernel())
    return _NC_CACHE


def _prep_in_maps(hidden_states, W_qkv, W_o):
    hid = np.asarray(hidden_states, dtype=np.float32).reshape(S, D)
    # packed [chunk, kp, p, j, 512] (kb = 2*kp+j pair-interleaved so a pair
    # transfer writes each partition as one contiguous 2KB run):
    # hidT[(2*kp+j)*128+p, n*512+c]
    hidT = np.ascontiguousarray(
        hid.T.reshape(NKB // 2, 2, 128, NCHUNK, 512).transpose(3, 0, 2, 1, 4)
    ).astype(np_bf16)
    W_qkv = np.asarray(W_qkv, dtype=np.float32)
    W_o = np.asarray(W_o, dtype=np.float32)

    inv = 1.0 / (THETA ** (np.arange(0, DH, 2, dtype=np.float64) / DH))
    fr = np.arange(S, dtype=np.float64)[:, None] * inv[None, :]  # [S, 64]
    cosT = np.cos(fr).T
    sinT = np.sin(fr).T
    cos2 = np.ascontiguousarray(np.concatenate([cosT, cosT], 0)).astype(np_bf16)
    sinS = np.ascontiguousarray(np.concatenate([-sinT, sinT], 0)).astype(np_bf16)

    in_maps = []
    for i in range(NCORES):
        q_cols = W_qkv[:, 512 * i:512 * i + 512]
        k_cols = W_qkv[:, H * DH + 128 * i:H * DH + 128 * i + 128]
        v_cols = W_qkv[:, (H + HKV) * DH + 128 * i:(H + HKV) * DH + 128 * i + 128]
        wkv_i = np.ascontiguousarray(
            np.concatenate([k_cols, v_cols], axis=1)
        ).astype(np_bf16)
        wq_i = np.ascontiguousarray(q_cols).astype(np_bf16)
        wo_i = np.ascontiguousarray(W_o[512 * i:512 * i + 512, :]).astype(np_bf16)
        in_maps.append(
            {
                "hidT": hidT,
                "wkv": wkv_i,
                "wq": wq_i,
                "wo": wo_i,
                "cos2": cos2,
                "sinS": sinS,
            }
        )
    return in_maps


def _run(in_maps, trace=False, **kw):
    nc = _get_nc()
    return run_bass_kernel_spmd(
        nc, in_maps, core_ids=list(range(NCORES)), trace=trace, **kw
    )


def _gather(res):
    total = np.zeros((S, D), dtype=np.float32)
    for i in range(NCORES):
        part = np.asarray(res.results[i]["out"], dtype=np.float32)
        # unpack [qb, dc, 128, 512] -> [S, D]
        total += part.transpose(0, 2, 1, 3).reshape(S, D)
    return total.reshape(S, B, D).astype(np.float32)


def kernel(hidden_states, sequence_mask, W_qkv, W_o):
    in_maps = _prep_in_maps(hidden_states, W_qkv, W_o)
    return _gather(_run(in_maps))



# revision 23
# speedup vs baseline: 1.1841x; 1.1841x over previous
"""Trainium2 Bass kernel for causal GQA self-attention (S=2048, D=4096, H=32,
HKV=8, DH=128), tensor-parallel over 8 NeuronCores.

Sharding: head-parallel TP. Core i owns q-heads [4i..4i+4) and kv-head i:
  - qkv_proj column shard -> q [S,512], k [S,128], v [S,128]
  - RoPE + causal attention for its 4 heads (GQA group shares the kv head)
  - o_proj row shard (rows [512i..512i+512)) -> bf16 partial [S, D]
Host sums the 8 partials (the "all-reduce") and reshapes to [S, 1, D].

v2 design ("S^T-direct"): attention scores are computed directly in kv-major
layout, sT[kv, q] = K·Q^T, using the dh-major K/Q slabs that the projection
already produces — this removes all 544 PE transposes of P and the 544
PSUM->SBUF copies that made the v1 pipeline DVE-bound and let HAM oscillate.
The softmax denominator l[q] = sum_kv exp(s) is computed on the tensor engine
with an all-ones [128,128] stationary operand: same N-stream cost as any MM,
and the result lands replicated on all 128 partitions, so the per-q reciprocal
can be applied to ctx^T with a single DVE multiply (no partition broadcast).

Per-core phases, interleaved per 512-row sequence chunk n (PE stays dense):
  1. qkv projection, 6 slabs of 128 cols (4q + k + v), N=512 streams;
     q/k slabs RoPE'd in place, v slab PE-transposed to seq-major tiles.
  2. attention for q-chunk c=n: per kv-tile t: sT MM (causal suffix only) ->
     diag mask add (DVE) -> exp (scalar, direct to SBUF bf16) -> l MM + PV MM,
     software-pipelined by 2 tiles so the PE never waits on the scalar exp.
  3. o_proj row shard for chunk c=n, N=512 streams, bf16 partial out.

All matmuls run in bf16 with fp32 PSUM accumulation. Softmax runs without
max-subtraction (logits are O(10) here, far inside fp32 exp range).
"""

import sys

sys.path.insert(0, "/opt/trn_rl_repo")

import numpy as np
import ml_dtypes
from contextlib import ExitStack

import concourse.bass as bass
import concourse.tile as tile
from concourse import mybir
from concourse.bass_utils import run_bass_kernel_spmd
from concourse.masks import make_identity

S, B, D = 2048, 1, 4096
H, HKV, DH = 32, 8, 128
NCORES = 8
HQ = H // HKV  # q heads per core = 4
NSLAB = HQ + 2  # 4 q slabs + k + v
THETA = 10000.0
SCALE = 1.0 / float(np.sqrt(DH))

BF16 = mybir.dt.bfloat16
F32 = mybir.dt.float32
np_bf16 = ml_dtypes.bfloat16

NKB = D // 128  # 32 contraction blocks for the projections
NQB = S // 128  # 16 seq blocks of 128
NCHUNK = S // 512  # 4 sequence chunks of 512


def build_kernel() -> bass.Bass:
    nc = bass.Bass()

    # hidT packed host-side as contiguous [chunk, kb, 128, 512] tiles so each
    # DMA is one dense 128KB block (strided 1KB rows run at ~half DMA rate)
    hidT_e = nc.declare_dram_parameter(
        "hidT", [NCHUNK, NKB // 2, 128, 2, 512], BF16, isOutput=False
    )
    # wqkv split into two contiguous tensors, cols [k|v] and [q0..q3], so the
    # ramp can deliver the k+v columns of a kb block (64KB) independently of
    # its q columns: the first matmul starts ~2.5us earlier and each kb
    # unlocks in 3 sub-deliveries instead of one 320KB step
    wkv_e = nc.declare_dram_parameter("wkv", [D, 2 * DH], BF16, isOutput=False)
    wq_e = nc.declare_dram_parameter("wq", [D, HQ * DH], BF16, isOutput=False)
    wo_e = nc.declare_dram_parameter("wo", [HQ * DH, D], BF16, isOutput=False)
    # cos2 = [cos; cos], sinS = [-sin; sin]  (dh-major halves stacked)
    cos_e = nc.declare_dram_parameter("cos2", [128, S], BF16, isOutput=False)
    sin_e = nc.declare_dram_parameter("sinS", [128, S], BF16, isOutput=False)
    # out packed [qb, dc, 128, 512]; host unpacks to [S, D]
    out_e = nc.declare_dram_parameter("out", [NQB, 8, 128, 512], BF16, isOutput=True)

    hidT = hidT_e[:]
    wkv = wkv_e[:]
    wq = wq_e[:]
    wo = wo_e[:]
    out = out_e[:]

    with tile.TileContext(nc) as tc, ExitStack() as ctx:
        singles = ctx.enter_context(tc.tile_pool(name="singles", bufs=1))

        # ---- persistent SBUF state ----
        wqkv_sb = singles.tile([128, NKB, NSLAB * DH], BF16)
        wo_sb = singles.tile([128, HQ, D], BF16)
        cos_sb = singles.tile([128, S], BF16)
        sin_sb = singles.tile([128, S], BF16)
        ident = singles.tile([128, 128], BF16)
        ones_sb = singles.tile([128, 128], BF16)
        maskT = singles.tile([128, 128], F32)
        # q slabs (m=0..3) + k slab (m=4), dh-major [dh, S], RoPE'd
        qkT_sb = singles.tile([128, HQ + 1, S], BF16)
        # V seq-major: tile t = rows [128t..128t+128) x [dh 128]
        v_sb = singles.tile([128, NQB, DH], BF16)
        # ctx^T per q-head slab [dh, S], softmax-normalized
        ctxT_sb = singles.tile([128, HQ, S], BF16)

        make_identity(nc, ident)
        nc.vector.memset(ones_sb, 1.0)
        # maskT[kv, q] = 0 where q >= kv (valid, diag incl), else -1e9
        nc.gpsimd.memset(maskT, 0.0)
        nc.gpsimd.affine_select(
            out=maskT,
            in_=maskT,
            compare_op=mybir.AluOpType.is_ge,
            fill=-1e9,
            base=0,
            pattern=[[1, 128]],
            channel_multiplier=-1,
        )

        with (
            tc.tile_pool(name="hidp", bufs=22) as hidp,
            tc.tile_pool(name="ropep", bufs=4) as ropep,
            tc.tile_pool(name="vtmp", bufs=2) as vtmpp,
            tc.tile_pool(name="ptp", bufs=4) as ptp,
            tc.tile_pool(name="paccp", bufs=2) as paccp,
            tc.tile_pool(name="linvp", bufs=2) as linvp,
            tc.tile_pool(name="outsb", bufs=4) as osp,
            # PSUM budget: 3 (mm: proj slabs + sT) + 1 (l) + 2 (ctx) + 2
            # (out: o_proj + v-transpose) = 8 banks
            tc.tile_pool(name="ps_mm", bufs=3, space="PSUM") as mmp,
            tc.tile_pool(name="ps_l", bufs=1, space="PSUM") as lpp,
            tc.tile_pool(name="ps_ctx", bufs=2, space="PSUM") as cpp,
            tc.tile_pool(name="ps_out", bufs=2, space="PSUM") as opp,
        ):
            # Deferred per-head softmax normalize: the final l matmul + the
            # Ln/Exp(-x) reciprocal + ctx multiply are emitted under PE cover
            # of the NEXT head's first score matmuls (or the next chunk's
            # first projection matmuls), so the exp->l-accumulate latency
            # never stalls the tensor engine.
            pending = [None]

            def flush_pending():
                if pending[0] is None:
                    return
                cc, hh, l_ps_, ctx_ps_, pacc_ = pending[0]
                pending[0] = None
                if pacc_ is not None:
                    nc.tensor.matmul(l_ps_, ones_sb, pacc_, start=True, stop=True)
                lnl = linvp.tile([128, 512], F32, name=f"lnl_{cc}_{hh}", tag="lnl")
                nc.scalar.activation(lnl, l_ps_, mybir.ActivationFunctionType.Ln)
                linv = linvp.tile(
                    [128, 512], F32, name=f"linv_{cc}_{hh}", tag="linv"
                )
                nc.scalar.activation(
                    linv, lnl, mybir.ActivationFunctionType.Exp, scale=-1.0
                )
                nc.vector.tensor_mul(
                    ctxT_sb[:, hh, cc * 512:(cc + 1) * 512], ctx_ps_, linv
                )

            def issue_ht(n):
                # paired DMAs (2 kb-blocks per transfer): the sync sequencer
                # dispatches each dma_start serially at ~0.6us, so transfer
                # count, not just bytes, gates chunk-0 delivery
                htp = []
                for kp in range(NKB // 2):
                    # first few transfers split into kb singles (interleaved
                    # kv-cols / ht / q-cols so the very first matmul's inputs
                    # land first); afterwards kb-pairs win on dispatch count
                    split = n == 0 and kp < 3
                    t_ = hidp.tile(
                        [128, 2, 512], BF16, name=f"ht_{n}_{kp}", tag="ht"
                    )
                    if n == 0:
                        if split:
                            for j in range(2):
                                kb = 2 * kp + j
                                nc.sync.dma_start(
                                    out=wqkv_sb[:, kb, 0:256],
                                    in_=wkv[kb * 128:(kb + 1) * 128, :],
                                )
                                nc.sync.dma_start(
                                    out=t_[:, j, :], in_=hidT[n, kp, :, j, :]
                                )
                                nc.sync.dma_start(
                                    out=wqkv_sb[:, kb, 256:768],
                                    in_=wq[kb * 128:(kb + 1) * 128, :],
                                )
                        else:
                            nc.sync.dma_start(
                                out=wqkv_sb[:, 2 * kp:2 * kp + 2, 0:256],
                                in_=wkv[kp * 256:(kp + 1) * 256, :].rearrange(
                                    "(j p) c -> p j c", p=128
                                ),
                            )
                            nc.sync.dma_start(
                                out=t_,
                                in_=hidT[n, kp],
                            )
                            nc.sync.dma_start(
                                out=wqkv_sb[:, 2 * kp:2 * kp + 2, 256:768],
                                in_=wq[kp * 256:(kp + 1) * 256, :].rearrange(
                                    "(j p) c -> p j c", p=128
                                ),
                            )
                        if kp == 2:
                            nc.sync.dma_start(out=cos_sb, in_=cos_e[:])
                            nc.sync.dma_start(out=sin_sb, in_=sin_e[:])
                    else:
                        nc.sync.dma_start(
                            out=t_,
                            in_=hidT[n, kp],
                        )
                    htp.append(t_)
                return htp

            htp_map = {0: issue_ht(0)}

            def make_chain(oc, dp_idx, ps_pool, ps_tag, fine_dma=False):
                iq, dp = dp_idx // 4, dp_idx % 4

                def chain():
                    qb = 4 * oc + iq
                    out_sb = osp.tile(
                        [128, 2, 512], BF16, name="out_sb", tag="out_sb"
                    )
                    for half in range(2):
                        dc = 2 * dp + half
                        out_ps = ps_pool.tile(
                            [128, 512], F32, name=f"out_ps_{qb}_{dc}",
                            tag=ps_tag,
                        )
                        for h in range(HQ):
                            nc.tensor.matmul(
                                out_ps,
                                ctxT_sb[:, h, qb * 128:(qb + 1) * 128],
                                wo_sb[:, h, dc * 512:(dc + 1) * 512],
                                start=(h == 0),
                                stop=(h == HQ - 1),
                            )
                        if half == 0:
                            nc.scalar.copy(out_sb[:, 0, :], out_ps)
                        else:
                            nc.vector.tensor_copy(out_sb[:, 1, :], out_ps)
                    if fine_dma:
                        for half in range(2):
                            nc.sync.dma_start(
                                out=out[qb, 2 * dp + half],
                                in_=out_sb[:, half, :],
                            )
                    else:
                        nc.sync.dma_start(
                            out=out[qb, 2 * dp:2 * dp + 2].rearrange(
                                "a p c -> p a c"
                            ),
                            in_=out_sb,
                        )

                return chain

            vts = {}

            def emit_proj(n):
                # ---- qkv projection for seq chunk n ----
                ht = [htp_map[n][kb // 2][:, kb % 2, :] for kb in range(NKB)]
                sl = slice(n * 512, (n + 1) * 512)
                # k slab first so its RoPE is long done when attention starts;
                # v 5th so its staging copy beats the h0 transposes;
                # (slab_idx in qkT_sb/v, column offset in wqkv)
                # All 6 slabs run kb-synchronously: hidp ring slots then
                # free progressively through the phase (slab-sequential order
                # frees them only during the last slab's sweep, compressing
                # the next chunk's hidT delivery into the phase tail), and at
                # chunk 0 the PE work unlocked per arriving (wkv, ht, wq)
                # kb-pair matches the DMA ramp's delivery rate.
                # Accumulator rings, chosen by when their previous occupant
                # frees: k first (ctx ring slot freed in the prior attention),
                # v/q0/q1 on the mm ring (sT slots), q3 on the l ring (freed
                # by the pending flush's early Ln), q2 last on the ctx ring
                # (freed by that flush's final DVE multiply, ~1.6us in).
                # Emission order also matches the ramp delivery order
                # kv-cols -> ht -> q-cols of each kb block.
                # order: the mm-ring slabs (v,q0,q1) finish first (their
                # PSUM slots are reused by the next attention's first sT
                # tiles within ~1us), then k (ctx slot, reused at ~+1.3us),
                # then q3/q2 whose borrowed l/ctx slots aren't touched for
                # 4-8us. The v slab leads so its staging copy is long done
                # before the attention-h0 transposes.
                SLABS = ((5, 128), (0, 256), (1, 384), (4, 0), (3, 640), (2, 512))
                groups = [SLABS]
                slab_pools = [mmp, mmp, mmp, cpp, lpp, cpp]
                slab_tags = ["mm", "mm", "mm", "ctx", "l", "ctx"]
                vt_box = [None]

                def finish_slab(m, ps, on_vector=False):
                    if m < NSLAB - 1:
                        # q or k slab: copy out of PSUM, then RoPE in place.
                        # The last two slabs to finish drain on the vector
                        # engine: their copies fire at phase end, and on the
                        # scalar engine they would queue in front of the next
                        # attention's exps (the attention critical path).
                        slab = qkT_sb[:, m, sl]
                        if on_vector:
                            nc.vector.tensor_copy(slab, ps)
                        else:
                            nc.scalar.copy(slab, ps)
                        rot = ropep.tile([128, 512], BF16, name="rot", tag="rot")
                        nc.sync.dma_start(out=rot[0:64, :], in_=qkT_sb[64:128, m, sl])
                        nc.sync.dma_start(out=rot[64:128, :], in_=qkT_sb[0:64, m, sl])
                        rt = ropep.tile([128, 512], BF16, name="rt", tag="rt")
                        nc.vector.tensor_mul(rt, rot, sin_sb[:, sl])
                        nc.vector.tensor_mul(slab, slab, cos_sb[:, sl])
                        nc.vector.tensor_add(slab, slab, rt)
                    else:
                        # v slab: stage to SBUF; PE-transposed to seq-major
                        # inside the attention A-loop (keeps PE dense)
                        vt_box[0] = vtmpp.tile([128, 512], BF16, name="vt", tag="vt")
                        nc.scalar.copy(vt_box[0], ps)

                grp = groups[0]
                pss = []
                for si, (m, _) in enumerate(grp):
                    pss.append(
                        slab_pools[si].tile(
                            [128, 512], F32, name=f"proj_ps_{n}_{m}",
                            tag=slab_tags[si],
                        )
                    )
                # last head's softmax normalize of the chunk-before-last.
                # Must precede the kb loop: q2's first matmul waits on the
                # ctx-ring slot this flush's DVE multiply frees, and that
                # multiply's l matmul must come first in the PE queue.
                flush_pending()
                # Each slab's sweep lags the previous by 2 kb: the six PSUM
                # drains then fire ~2.5us apart, each overlapped by the later
                # slabs' remaining matmuls, instead of bunching at phase end
                # in front of the next attention's exps in the scalar queue.
                # It also gives the ramp's q-piece deliveries (chunk 0) 4+
                # kb of slack behind their kv/ht pieces, and the borrowed
                # l/ctx-ring slots (q3/q2) time to clear the pending flush.
                LAGS = (0, 2, 4, 6, 8, 10)
                for step in range(NKB + LAGS[-1] + 1):
                    for gi, (m, coff) in enumerate(grp):
                        kb_g = step - LAGS[gi]
                        if not (0 <= kb_g < NKB):
                            continue
                        nc.tensor.matmul(
                            pss[gi],
                            wqkv_sb[:, kb_g, coff:coff + 128],
                            ht[kb_g],
                            start=(kb_g == 0),
                            stop=(kb_g == NKB - 1),
                        )
                        if kb_g == NKB - 1:
                            # this slab is done: drain + RoPE it now, under
                            # cover of the remaining slabs' matmuls
                            finish_slab(m, pss[gi])
                    if step == NKB - 1 and n + 1 < NCHUNK:
                        # prefetch next chunk's hidden states (most finishes
                        # already emitted; their rot DMAs never head-of-line-
                        # block these dispatches for long)
                        htp_map[n + 1] = issue_ht(n + 1)
                vts[n] = vt_box[0]
                if n == 0:
                    # o_proj weights: first needed by the chains in attn c1
                    for h in range(HQ):
                        nc.sync.dma_start(
                            out=wo_sb[:, h, :], in_=wo[h * 128:(h + 1) * 128, :]
                        )

            def emit_attention(c):
                # ---- attention for q-chunk c, with o_proj chains for chunk
                # c-1 interleaved between score tiles. The attention inner
                # loop is scalar-bound (exp of a [128,512] tile ~530ns vs
                # ~432ns of PE per tile), so without filler the PE idles
                # ~100ns/tile waiting on exp; each interleaved chain adds
                # ~1.7us of exp-independent PE work.
                vt = vts.pop(c)
                chains = []
                if c >= 1:
                    chains = [
                        make_chain(c - 1, k, opp, "out") for k in range(16)
                    ]
                ci = [0]

                def emit_chain():
                    if ci[0] < len(chains):
                        chains[ci[0]]()
                        ci[0] += 1

                ntile = 4 * (c + 1)
                slots = HQ * ntile
                stride = max(1, slots // 16)
                slot = [0]
                for h in range(HQ):
                    # (t, qoff, w): kv tile t covers seq [128t, 128t+128); for
                    # diagonal tiles only q >= 128t attends -> stream suffix
                    tiles = []
                    for t in range(ntile):
                        qoff = max(0, 128 * (t - 4 * c))
                        tiles.append((t, qoff, 512 - qoff))
                    nt = len(tiles)
                    l_ps = lpp.tile([128, 512], F32, name=f"l_ps_{c}_{h}", tag="l")
                    ctx_ps = cpp.tile(
                        [128, 512], F32, name=f"ctx_ps_{c}_{h}", tag="ctx"
                    )
                    # c>=1: sum pT tiles on DVE; a single ones-matmul in the
                    # flush then reduces over kv (one PE stream instead of one
                    # per tile). c=0 keeps the per-tile ones-matmuls: A0 has
                    # no o_proj chains to fill the exp-bound stretches, and
                    # the l matmuls (which run after each exp) are free PE
                    # fill there.
                    if c >= 1:
                        pacc = paccp.tile(
                            [128, 512], BF16, name=f"pacc_{c}_{h}", tag="pacc"
                        )
                        nc.vector.memset(pacc, 0.0)
                    else:
                        pacc = None
                    pT_tiles = {}

                    def stage_a(i):
                        t, qoff, w = tiles[i]
                        sT = mmp.tile(
                            [128, 512], F32, name=f"sT_{c}_{h}_{t}", tag="mm"
                        )
                        nc.tensor.matmul(
                            sT[:, 0:w],
                            qkT_sb[:, HQ, t * 128:(t + 1) * 128],
                            qkT_sb[:, h, c * 512 + qoff:(c + 1) * 512],
                            start=True,
                            stop=True,
                        )
                        if t >= 4 * c:
                            # diagonal tile: first 128 streamed q's need mask
                            nc.vector.tensor_add(sT[:, 0:128], sT[:, 0:128], maskT)
                        pT = ptp.tile([128, 512], BF16, name=f"pT_{c}_{h}_{t}", tag="pt")
                        nc.scalar.activation(
                            pT[:, 0:w],
                            sT[:, 0:w],
                            mybir.ActivationFunctionType.Exp,
                            scale=SCALE,
                        )
                        pT_tiles[i] = pT
                        if pacc is not None:
                            # accumulate sum-over-tiles on DVE; the single
                            # all-ones matmul in the flush reduces over kv
                            nc.vector.tensor_add(
                                pacc[:, qoff:512], pacc[:, qoff:512], pT[:, 0:w]
                            )

                    def stage_c(i):
                        t, qoff, w = tiles[i]
                        pT = pT_tiles.pop(i)
                        first, last = i == 0, i == nt - 1
                        if pacc is None:
                            nc.tensor.matmul(
                                l_ps[:, qoff:512],
                                ones_sb,
                                pT[:, 0:w],
                                start=first,
                                stop=last,
                            )
                        nc.tensor.matmul(
                            ctx_ps[:, qoff:512],
                            v_sb[:, t, :],
                            pT[:, 0:w],
                            start=first,
                            stop=last,
                        )

                    DEPTH = 2
                    vt0 = max(0, 4 * c - 4)
                    for i in range(nt):
                        stage_a(i)
                        if h == 0 and vt0 <= i < vt0 + 4:
                            # chunk c's v tiles: dh-major -> seq-major, first
                            # needed by stage_c(4c); scheduled 4 tiles early
                            # so the DVE copies run mid-head, clear of the
                            # head-start rope/chain burst on DVE
                            j = i - vt0
                            vtp = opp.tile([128, 128], BF16, name="vtp", tag="out")
                            nc.tensor.transpose(
                                vtp, vt[:, j * 128:(j + 1) * 128], ident
                            )
                            nc.vector.tensor_copy(v_sb[:, 4 * c + j, :], vtp)
                        if i == 1:
                            flush_pending()
                        if i >= DEPTH:
                            stage_c(i - DEPTH)
                        slot[0] += 1
                        if slot[0] % stride == 1 or stride == 1:
                            emit_chain()
                    for i in range(max(0, nt - DEPTH), nt):
                        stage_c(i)
                    pending[0] = (c, h, l_ps, ctx_ps, pacc)
                while ci[0] < len(chains):
                    emit_chain()

            # phase order P0 P1 A0 P2 A1 P3 A2 A3 tail: attention c runs
            # after proj c+1, whose matmuls cover chunk c's PSUM drains and
            # RoPE chains (otherwise they serialize at the boundary)
            for n in range(NCHUNK):
                emit_proj(n)
                if n >= 1:
                    emit_attention(n - 1)
            flush_pending()
            emit_attention(NCHUNK - 1)

            # ---- tail: o_proj for the final chunk. The attention mm ring
            # (3 PSUM banks) is idle now; alternating out_ps between it and
            # the out ring gives a 5-deep ring so the PE never waits on the
            # scalar/vector PSUM drains.
            flush_pending()
            for k in range(16):
                pool, tag = ((mmp, "mm"), (opp, "out"))[k % 2]
                make_chain(NCHUNK - 1, k, pool, tag, fine_dma=(k >= 14))()
            flush_pending()

    return nc


def _legalize_waits(j):
    """Split multi-wait instructions: the TPB ISA gives each instruction (and
    each dynamic-DMA descriptor) a single semaphore-wait slot, and this walrus
    build errors on extras instead of splitting them. Hoist all but one wait
    into standalone EventSemaphore instructions on the issuing engine, placed
    immediately before the instruction (engine streams execute in program
    order, so the waits complete before the op issues / the descriptor posts).
    """
    n_new = 0
    for fn in j["functions"]:
        for bb in fn["blocks"]:
            insts = bb.get("instructions", [])
            out = []
            for inst in insts:
                si = inst.get("sync_info") or {}
                waits = si.get("on_wait") or []
                if len(waits) > 1:
                    for w in waits[:-1]:
                        n_new += 1
                        out.append(
                            {
                                "name": f"{inst['name']}-lw{n_new}",
                                "opcode": "EventSemaphore",
                                "engine": inst["engine"],
                                "ins": [],
                                "outs": [],
                                "debug": inst.get("debug"),
                                "sync_info": {"on_update": [], "on_wait": [w]},
                            }
                        )
                    si = dict(si)
                    si["on_wait"] = [waits[-1]]
                    inst = dict(inst)
                    inst["sync_info"] = si
                out.append(inst)
            bb["instructions"] = out
    return j


def _patch_json(nc):
    import json

    orig = nc.to_json_bytes

    def patched():
        j = json.loads(orig())
        return json.dumps(_legalize_waits(j)).encode()

    nc.to_json_bytes = patched
    return nc


_NC_CACHE = None


def _get_nc():
    global _NC_CACHE
    if _NC_CACHE is None:
        _NC_CACHE = _patch_json(build_kernel())
    return _NC_CACHE


def _prep_in_maps(hidden_states, W_qkv, W_o):
    hid = np.asarray(hidden_states, dtype=np.float32).reshape(S, D)
    # packed [chunk, kp, p, j, 512] (kb = 2*kp+j pair-interleaved so a pair
    # transfer writes each partition as one contiguous 2KB run):
    # hidT[(2*kp+j)*128+p, n*512+c]
    hidT = np.ascontiguousarray(
        hid.T.reshape(NKB // 2, 2, 128, NCHUNK, 512).transpose(3, 0, 2, 1, 4)
    ).astype(np_bf16)
    W_qkv = np.asarray(W_qkv, dtype=np.float32)
    W_o = np.asarray(W_o, dtype=np.float32)

    inv = 1.0 / (THETA ** (np.arange(0, DH, 2, dtype=np.float64) / DH))
    fr = np.arange(S, dtype=np.float64)[:, None] * inv[None, :]  # [S, 64]
    cosT = np.cos(fr).T
    sinT = np.sin(fr).T
    cos2 = np.ascontiguousarray(np.concatenate([cosT, cosT], 0)).astype(np_bf16)
    sinS = np.ascontiguousarray(np.concatenate([-sinT, sinT], 0)).astype(np_bf16)

    in_maps = []
    for i in range(NCORES):
        q_cols = W_qkv[:, 512 * i:512 * i + 512]
        k_cols = W_qkv[:, H * DH + 128 * i:H * DH + 128 * i + 128]
        v_cols = W_qkv[:, (H + HKV) * DH + 128 * i:(H + HKV) * DH + 128 * i + 128]
        wkv_i = np.ascontiguousarray(
            np.concatenate([k_cols, v_cols], axis=1)
        ).astype(np_bf16)
        wq_i = np.ascontiguousarray(q_cols).astype(np_bf16)
        wo_i = np.ascontiguousarray(W_o[512 * i:512 * i + 512, :]).astype(np_bf16)
        in_maps.append(
            {
                "hidT": hidT,
                "wkv": wkv_i,
                "wq": wq_i,
                "wo": wo_i,
                "cos2": cos2,
                "sinS": sinS,
            }
        )
    return in_maps


def _run(in_maps, trace=False, **kw):
    nc = _get_nc()
    return run_bass_kernel_spmd(
        nc, in_maps, core_ids=list(range(NCORES)), trace=trace, **kw
    )


def _gather(res):
    total = np.zeros((S, D), dtype=np.float32)
    for i in range(NCORES):
        part = np.asarray(res.results[i]["out"], dtype=np.float32)
        # unpack [qb, dc, 128, 512] -> [S, D]
        total += part.transpose(0, 2, 1, 3).reshape(S, D)
    return total.reshape(S, B, D).astype(np.float32)


def kernel(hidden_states, sequence_mask, W_qkv, W_o):
    in_maps = _prep_in_maps(hidden_states, W_qkv, W_o)
    return _gather(_run(in_maps))



# revision 24
# speedup vs baseline: 1.1937x; 1.0081x over previous
"""Trainium2 Bass kernel for causal GQA self-attention (S=2048, D=4096, H=32,
HKV=8, DH=128), tensor-parallel over 8 NeuronCores.

Sharding: head-parallel TP. Core i owns q-heads [4i..4i+4) and kv-head i:
  - qkv_proj column shard -> q [S,512], k [S,128], v [S,128]
  - RoPE + causal attention for its 4 heads (GQA group shares the kv head)
  - o_proj row shard (rows [512i..512i+512)) -> bf16 partial [S, D]
Host sums the 8 partials (the "all-reduce") and reshapes to [S, 1, D].

v2 design ("S^T-direct"): attention scores are computed directly in kv-major
layout, sT[kv, q] = K·Q^T, using the dh-major K/Q slabs that the projection
already produces — this removes all 544 PE transposes of P and the 544
PSUM->SBUF copies that made the v1 pipeline DVE-bound and let HAM oscillate.
The softmax denominator l[q] = sum_kv exp(s) is computed on the tensor engine
with an all-ones [128,128] stationary operand: same N-stream cost as any MM,
and the result lands replicated on all 128 partitions, so the per-q reciprocal
can be applied to ctx^T with a single DVE multiply (no partition broadcast).

Per-core phases, interleaved per 512-row sequence chunk n (PE stays dense):
  1. qkv projection, 6 slabs of 128 cols (4q + k + v), N=512 streams;
     q/k slabs RoPE'd in place, v slab PE-transposed to seq-major tiles.
  2. attention for q-chunk c=n: per kv-tile t: sT MM (causal suffix only) ->
     diag mask add (DVE) -> exp (scalar, direct to SBUF bf16) -> l MM + PV MM,
     software-pipelined by 2 tiles so the PE never waits on the scalar exp.
  3. o_proj row shard for chunk c=n, N=512 streams, bf16 partial out.

All matmuls run in bf16 with fp32 PSUM accumulation. Softmax runs without
max-subtraction (logits are O(10) here, far inside fp32 exp range).
"""

import sys

sys.path.insert(0, "/opt/trn_rl_repo")

import numpy as np
import ml_dtypes
from contextlib import ExitStack

import concourse.bass as bass
import concourse.tile as tile
from concourse import mybir
from concourse.bass_utils import run_bass_kernel_spmd
from concourse.masks import make_identity

S, B, D = 2048, 1, 4096
H, HKV, DH = 32, 8, 128
NCORES = 8
HQ = H // HKV  # q heads per core = 4
NSLAB = HQ + 2  # 4 q slabs + k + v
THETA = 10000.0
SCALE = 1.0 / float(np.sqrt(DH))

BF16 = mybir.dt.bfloat16
F32 = mybir.dt.float32
np_bf16 = ml_dtypes.bfloat16

NKB = D // 128  # 32 contraction blocks for the projections
NQB = S // 128  # 16 seq blocks of 128
NCHUNK = S // 512  # 4 sequence chunks of 512


def build_kernel() -> bass.Bass:
    nc = bass.Bass()

    # hidT packed host-side as contiguous [chunk, kb, 128, 512] tiles so each
    # DMA is one dense 128KB block (strided 1KB rows run at ~half DMA rate)
    hidT_e = nc.declare_dram_parameter(
        "hidT", [NCHUNK, NKB // 2, 128, 2, 512], BF16, isOutput=False
    )
    # wqkv split into two contiguous tensors, cols [k|v] and [q0..q3], so the
    # ramp can deliver the k+v columns of a kb block (64KB) independently of
    # its q columns: the first matmul starts ~2.5us earlier and each kb
    # unlocks in 3 sub-deliveries instead of one 320KB step
    wkv_e = nc.declare_dram_parameter("wkv", [D, 2 * DH], BF16, isOutput=False)
    wq_e = nc.declare_dram_parameter("wq", [D, HQ * DH], BF16, isOutput=False)
    wo_e = nc.declare_dram_parameter("wo", [HQ * DH, D], BF16, isOutput=False)
    # cos2 = [cos; cos], sinS = [-sin; sin]  (dh-major halves stacked)
    cos_e = nc.declare_dram_parameter("cos2", [128, S], BF16, isOutput=False)
    sin_e = nc.declare_dram_parameter("sinS", [128, S], BF16, isOutput=False)
    # out packed [qb, dc, 128, 512]; host unpacks to [S, D]
    out_e = nc.declare_dram_parameter("out", [NQB, 8, 128, 512], BF16, isOutput=True)

    hidT = hidT_e[:]
    wkv = wkv_e[:]
    wq = wq_e[:]
    wo = wo_e[:]
    out = out_e[:]

    with tile.TileContext(nc) as tc, ExitStack() as ctx:
        singles = ctx.enter_context(tc.tile_pool(name="singles", bufs=1))

        # ---- persistent SBUF state ----
        wqkv_sb = singles.tile([128, NKB, NSLAB * DH], BF16)
        wo_sb = singles.tile([128, HQ, D], BF16)
        cos_sb = singles.tile([128, S], BF16)
        sin_sb = singles.tile([128, S], BF16)
        ident = singles.tile([128, 128], BF16)
        ones_sb = singles.tile([128, 128], BF16)
        maskT = singles.tile([128, 128], F32)
        # q slabs (m=0..3) + k slab (m=4), dh-major [dh, S], RoPE'd
        qkT_sb = singles.tile([128, HQ + 1, S], BF16)
        # V seq-major: tile t = rows [128t..128t+128) x [dh 128]
        v_sb = singles.tile([128, NQB, DH], BF16)
        # ctx^T per q-head slab [dh, S], softmax-normalized
        ctxT_sb = singles.tile([128, HQ, S], BF16)

        make_identity(nc, ident)
        nc.vector.memset(ones_sb, 1.0)
        # maskT[kv, q] = 0 where q >= kv (valid, diag incl), else -1e9
        nc.gpsimd.memset(maskT, 0.0)
        nc.gpsimd.affine_select(
            out=maskT,
            in_=maskT,
            compare_op=mybir.AluOpType.is_ge,
            fill=-1e9,
            base=0,
            pattern=[[1, 128]],
            channel_multiplier=-1,
        )

        with (
            tc.tile_pool(name="hidp", bufs=22) as hidp,
            tc.tile_pool(name="ropep", bufs=4) as ropep,
            tc.tile_pool(name="vtmp", bufs=2) as vtmpp,
            tc.tile_pool(name="ptp", bufs=4) as ptp,
            tc.tile_pool(name="paccp", bufs=2) as paccp,
            tc.tile_pool(name="linvp", bufs=2) as linvp,
            tc.tile_pool(name="outsb", bufs=4) as osp,
            # PSUM budget: 3 (mm: proj slabs + sT) + 1 (l) + 2 (ctx) + 2
            # (out: o_proj + v-transpose) = 8 banks
            tc.tile_pool(name="ps_mm", bufs=3, space="PSUM") as mmp,
            tc.tile_pool(name="ps_l", bufs=1, space="PSUM") as lpp,
            tc.tile_pool(name="ps_ctx", bufs=2, space="PSUM") as cpp,
            tc.tile_pool(name="ps_out", bufs=2, space="PSUM") as opp,
        ):
            # Deferred per-head softmax normalize: the final l matmul + the
            # Ln/Exp(-x) reciprocal + ctx multiply are emitted under PE cover
            # of the NEXT head's first score matmuls (or the next chunk's
            # first projection matmuls), so the exp->l-accumulate latency
            # never stalls the tensor engine.
            pending = [None]

            def flush_pending():
                if pending[0] is None:
                    return
                cc, hh, l_ps_, ctx_ps_, pacc_ = pending[0]
                pending[0] = None
                if pacc_ is not None:
                    nc.tensor.matmul(l_ps_, ones_sb, pacc_, start=True, stop=True)
                lnl = linvp.tile([128, 512], F32, name=f"lnl_{cc}_{hh}", tag="lnl")
                nc.scalar.activation(lnl, l_ps_, mybir.ActivationFunctionType.Ln)
                linv = linvp.tile(
                    [128, 512], F32, name=f"linv_{cc}_{hh}", tag="linv"
                )
                nc.scalar.activation(
                    linv, lnl, mybir.ActivationFunctionType.Exp, scale=-1.0
                )
                nc.vector.tensor_mul(
                    ctxT_sb[:, hh, cc * 512:(cc + 1) * 512], ctx_ps_, linv
                )

            def issue_ht(n):
                # paired DMAs (2 kb-blocks per transfer): the sync sequencer
                # dispatches each dma_start serially at ~0.6us, so transfer
                # count, not just bytes, gates chunk-0 delivery
                htp = []
                for kp in range(NKB // 2):
                    # first few transfers split into kb singles (interleaved
                    # kv-cols / ht / q-cols so the very first matmul's inputs
                    # land first); afterwards kb-pairs win on dispatch count
                    split = n == 0 and kp < 3
                    t_ = hidp.tile(
                        [128, 2, 512], BF16, name=f"ht_{n}_{kp}", tag="ht"
                    )
                    if n == 0:
                        if split:
                            for j in range(2):
                                kb = 2 * kp + j
                                nc.sync.dma_start(
                                    out=wqkv_sb[:, kb, 0:256],
                                    in_=wkv[kb * 128:(kb + 1) * 128, :],
                                )
                                nc.sync.dma_start(
                                    out=t_[:, j, :], in_=hidT[n, kp, :, j, :]
                                )
                                nc.sync.dma_start(
                                    out=wqkv_sb[:, kb, 256:768],
                                    in_=wq[kb * 128:(kb + 1) * 128, :],
                                )
                        else:
                            nc.sync.dma_start(
                                out=wqkv_sb[:, 2 * kp:2 * kp + 2, 0:256],
                                in_=wkv[kp * 256:(kp + 1) * 256, :].rearrange(
                                    "(j p) c -> p j c", p=128
                                ),
                            )
                            nc.sync.dma_start(
                                out=t_,
                                in_=hidT[n, kp],
                            )
                            nc.sync.dma_start(
                                out=wqkv_sb[:, 2 * kp:2 * kp + 2, 256:768],
                                in_=wq[kp * 256:(kp + 1) * 256, :].rearrange(
                                    "(j p) c -> p j c", p=128
                                ),
                            )
                        if kp == 2:
                            nc.sync.dma_start(out=cos_sb, in_=cos_e[:])
                            nc.sync.dma_start(out=sin_sb, in_=sin_e[:])
                    else:
                        nc.sync.dma_start(
                            out=t_,
                            in_=hidT[n, kp],
                        )
                    htp.append(t_)
                return htp

            htp_map = {0: issue_ht(0)}

            def make_chain(oc, dp_idx, ps_pool, ps_tag, fine_dma=False):
                iq, dp = dp_idx // 4, dp_idx % 4

                def chain():
                    qb = 4 * oc + iq
                    out_sb = osp.tile(
                        [128, 2, 512], BF16, name="out_sb", tag="out_sb"
                    )
                    for half in range(2):
                        dc = 2 * dp + half
                        out_ps = ps_pool.tile(
                            [128, 512], F32, name=f"out_ps_{qb}_{dc}",
                            tag=ps_tag,
                        )
                        for h in range(HQ):
                            nc.tensor.matmul(
                                out_ps,
                                ctxT_sb[:, h, qb * 128:(qb + 1) * 128],
                                wo_sb[:, h, dc * 512:(dc + 1) * 512],
                                start=(h == 0),
                                stop=(h == HQ - 1),
                            )
                        if half == 0:
                            nc.scalar.copy(out_sb[:, 0, :], out_ps)
                        else:
                            nc.vector.tensor_copy(out_sb[:, 1, :], out_ps)
                    if fine_dma:
                        for half in range(2):
                            nc.sync.dma_start(
                                out=out[qb, 2 * dp + half],
                                in_=out_sb[:, half, :],
                            )
                    else:
                        nc.sync.dma_start(
                            out=out[qb, 2 * dp:2 * dp + 2].rearrange(
                                "a p c -> p a c"
                            ),
                            in_=out_sb,
                        )

                return chain

            vts = {}
            deferred_fin = []

            def emit_proj(n):
                # ---- qkv projection for seq chunk n ----
                ht = [htp_map[n][kb // 2][:, kb % 2, :] for kb in range(NKB)]
                sl = slice(n * 512, (n + 1) * 512)
                # k slab first so its RoPE is long done when attention starts;
                # v 5th so its staging copy beats the h0 transposes;
                # (slab_idx in qkT_sb/v, column offset in wqkv)
                # All 6 slabs run kb-synchronously: hidp ring slots then
                # free progressively through the phase (slab-sequential order
                # frees them only during the last slab's sweep, compressing
                # the next chunk's hidT delivery into the phase tail), and at
                # chunk 0 the PE work unlocked per arriving (wkv, ht, wq)
                # kb-pair matches the DMA ramp's delivery rate.
                # Accumulator rings, chosen by when their previous occupant
                # frees: k first (ctx ring slot freed in the prior attention),
                # v/q0/q1 on the mm ring (sT slots), q3 on the l ring (freed
                # by the pending flush's early Ln), q2 last on the ctx ring
                # (freed by that flush's final DVE multiply, ~1.6us in).
                # Emission order also matches the ramp delivery order
                # kv-cols -> ht -> q-cols of each kb block.
                # order: the mm-ring slabs (v,q0,q1) finish first (their
                # PSUM slots are reused by the next attention's first sT
                # tiles within ~1us), then k (ctx slot, reused at ~+1.3us),
                # then q3/q2 whose borrowed l/ctx slots aren't touched for
                # 4-8us. The v slab leads so its staging copy is long done
                # before the attention-h0 transposes.
                SLABS = ((5, 128), (0, 256), (1, 384), (4, 0), (3, 640), (2, 512))
                groups = [SLABS]
                slab_pools = [mmp, mmp, mmp, cpp, lpp, cpp]
                slab_tags = ["mm", "mm", "mm", "ctx", "l", "ctx"]
                vt_box = [None]

                def finish_slab(m, ps, fn=None):
                    fsl = sl if fn is None else slice(fn * 512, (fn + 1) * 512)
                    if m < NSLAB - 1:
                        # q or k slab: copy out of PSUM, then RoPE in place
                        slab = qkT_sb[:, m, fsl]
                        nc.scalar.copy(slab, ps)
                        rot = ropep.tile([128, 512], BF16, name="rot", tag="rot")
                        nc.sync.dma_start(out=rot[0:64, :], in_=qkT_sb[64:128, m, fsl])
                        nc.sync.dma_start(out=rot[64:128, :], in_=qkT_sb[0:64, m, fsl])
                        rt = ropep.tile([128, 512], BF16, name="rt", tag="rt")
                        nc.vector.tensor_mul(rt, rot, sin_sb[:, fsl])
                        nc.vector.tensor_mul(slab, slab, cos_sb[:, fsl])
                        nc.vector.tensor_add(slab, slab, rt)
                    else:
                        # v slab: stage to SBUF; PE-transposed to seq-major
                        # inside the attention A-loop (keeps PE dense)
                        vt_box[0] = vtmpp.tile([128, 512], BF16, name="vt", tag="vt")
                        nc.scalar.copy(vt_box[0], ps)

                grp = groups[0]
                pss = []
                for si, (m, _) in enumerate(grp):
                    pss.append(
                        slab_pools[si].tile(
                            [128, 512], F32, name=f"proj_ps_{n}_{m}",
                            tag=slab_tags[si],
                        )
                    )
                # last head's softmax normalize of the chunk-before-last.
                # Must precede the kb loop: q2's first matmul waits on the
                # ctx-ring slot this flush's DVE multiply frees, and that
                # multiply's l matmul must come first in the PE queue.
                flush_pending()
                # Each slab's sweep lags the previous by 2 kb: the six PSUM
                # drains then fire ~2.5us apart, each overlapped by the later
                # slabs' remaining matmuls, instead of bunching at phase end
                # in front of the next attention's exps in the scalar queue.
                # It also gives the ramp's q-piece deliveries (chunk 0) 4+
                # kb of slack behind their kv/ht pieces, and the borrowed
                # l/ctx-ring slots (q3/q2) time to clear the pending flush.
                LAGS = (0, 2, 4, 6, 8, 10)
                for step in range(NKB + LAGS[-1] + 1):
                    for gi, (m, coff) in enumerate(grp):
                        kb_g = step - LAGS[gi]
                        if not (0 <= kb_g < NKB):
                            continue
                        nc.tensor.matmul(
                            pss[gi],
                            wqkv_sb[:, kb_g, coff:coff + 128],
                            ht[kb_g],
                            start=(kb_g == 0),
                            stop=(kb_g == NKB - 1),
                        )
                        if kb_g == NKB - 1:
                            if n >= 1 and gi >= 4:
                                # q3/q2 finish at phase end; their scalar
                                # copies would queue in front of the next
                                # attention's first exps (its critical path),
                                # and their PSUM slots/RoPE results are not
                                # needed for another 4-8us -- defer them into
                                # that attention's head 0
                                deferred_fin.append(
                                    (lambda mm, pp, nn: lambda: finish_slab(
                                        mm, pp, fn=nn
                                    ))(m, pss[gi], n)
                                )
                            else:
                                # slab done: drain + RoPE it now, under cover
                                # of the remaining slabs' matmuls
                                finish_slab(m, pss[gi])
                    if step == NKB - 1 and n + 1 < NCHUNK:
                        # prefetch next chunk's hidden states (most finishes
                        # already emitted; their rot DMAs never head-of-line-
                        # block these dispatches for long)
                        htp_map[n + 1] = issue_ht(n + 1)
                vts[n] = vt_box[0]
                if n == 0:
                    # o_proj weights: first needed by the chains in attn c1
                    for h in range(HQ):
                        nc.sync.dma_start(
                            out=wo_sb[:, h, :], in_=wo[h * 128:(h + 1) * 128, :]
                        )

            def emit_attention(c):
                # ---- attention for q-chunk c, with o_proj chains for chunk
                # c-1 interleaved between score tiles. The attention inner
                # loop is scalar-bound (exp of a [128,512] tile ~530ns vs
                # ~432ns of PE per tile), so without filler the PE idles
                # ~100ns/tile waiting on exp; each interleaved chain adds
                # ~1.7us of exp-independent PE work.
                vt = vts.pop(c)
                chains = []
                if c >= 1:
                    chains = [
                        make_chain(c - 1, k, opp, "out") for k in range(16)
                    ]
                ci = [0]

                def emit_chain():
                    if ci[0] < len(chains):
                        chains[ci[0]]()
                        ci[0] += 1

                ntile = 4 * (c + 1)
                slots = HQ * ntile
                stride = max(1, slots // 16)
                slot = [0]
                for h in range(HQ):
                    # (t, qoff, w): kv tile t covers seq [128t, 128t+128); for
                    # diagonal tiles only q >= 128t attends -> stream suffix
                    tiles = []
                    for t in range(ntile):
                        qoff = max(0, 128 * (t - 4 * c))
                        tiles.append((t, qoff, 512 - qoff))
                    nt = len(tiles)
                    l_ps = lpp.tile([128, 512], F32, name=f"l_ps_{c}_{h}", tag="l")
                    ctx_ps = cpp.tile(
                        [128, 512], F32, name=f"ctx_ps_{c}_{h}", tag="ctx"
                    )
                    # c>=1: sum pT tiles on DVE; a single ones-matmul in the
                    # flush then reduces over kv (one PE stream instead of one
                    # per tile). c=0 keeps the per-tile ones-matmuls: A0 has
                    # no o_proj chains to fill the exp-bound stretches, and
                    # the l matmuls (which run after each exp) are free PE
                    # fill there.
                    if c >= 1:
                        pacc = paccp.tile(
                            [128, 512], BF16, name=f"pacc_{c}_{h}", tag="pacc"
                        )
                        nc.vector.memset(pacc, 0.0)
                    else:
                        pacc = None
                    pT_tiles = {}

                    def stage_a(i):
                        t, qoff, w = tiles[i]
                        sT = mmp.tile(
                            [128, 512], F32, name=f"sT_{c}_{h}_{t}", tag="mm"
                        )
                        nc.tensor.matmul(
                            sT[:, 0:w],
                            qkT_sb[:, HQ, t * 128:(t + 1) * 128],
                            qkT_sb[:, h, c * 512 + qoff:(c + 1) * 512],
                            start=True,
                            stop=True,
                        )
                        if t >= 4 * c:
                            # diagonal tile: first 128 streamed q's need mask
                            nc.vector.tensor_add(sT[:, 0:128], sT[:, 0:128], maskT)
                        pT = ptp.tile([128, 512], BF16, name=f"pT_{c}_{h}_{t}", tag="pt")
                        nc.scalar.activation(
                            pT[:, 0:w],
                            sT[:, 0:w],
                            mybir.ActivationFunctionType.Exp,
                            scale=SCALE,
                        )
                        pT_tiles[i] = pT
                        if pacc is not None:
                            # accumulate sum-over-tiles on DVE; the single
                            # all-ones matmul in the flush reduces over kv
                            nc.vector.tensor_add(
                                pacc[:, qoff:512], pacc[:, qoff:512], pT[:, 0:w]
                            )

                    def stage_c(i):
                        t, qoff, w = tiles[i]
                        pT = pT_tiles.pop(i)
                        first, last = i == 0, i == nt - 1
                        if pacc is None:
                            nc.tensor.matmul(
                                l_ps[:, qoff:512],
                                ones_sb,
                                pT[:, 0:w],
                                start=first,
                                stop=last,
                            )
                        nc.tensor.matmul(
                            ctx_ps[:, qoff:512],
                            v_sb[:, t, :],
                            pT[:, 0:w],
                            start=first,
                            stop=last,
                        )

                    DEPTH = 2
                    vt0 = max(0, 4 * c - 4)
                    for i in range(nt):
                        stage_a(i)
                        if h == 0 and vt0 <= i < vt0 + 4:
                            # chunk c's v tiles: dh-major -> seq-major, first
                            # needed by stage_c(4c); scheduled 4 tiles early
                            # so the DVE copies run mid-head, clear of the
                            # head-start rope/chain burst on DVE
                            j = i - vt0
                            vtp = opp.tile([128, 128], BF16, name="vtp", tag="out")
                            nc.tensor.transpose(
                                vtp, vt[:, j * 128:(j + 1) * 128], ident
                            )
                            nc.vector.tensor_copy(v_sb[:, 4 * c + j, :], vtp)
                        if i == 1:
                            flush_pending()
                        if h == 0 and i in (2, 3) and deferred_fin:
                            # drain a deferred proj slab now: its scalar copy
                            # queues behind this head's first exps
                            deferred_fin.pop(0)()
                        if i >= DEPTH:
                            stage_c(i - DEPTH)
                        slot[0] += 1
                        if slot[0] % stride == 1 or stride == 1:
                            emit_chain()
                    for i in range(max(0, nt - DEPTH), nt):
                        stage_c(i)
                    pending[0] = (c, h, l_ps, ctx_ps, pacc)
                while ci[0] < len(chains):
                    emit_chain()

            # phase order P0 P1 A0 P2 A1 P3 A2 A3 tail: attention c runs
            # after proj c+1, whose matmuls cover chunk c's PSUM drains and
            # RoPE chains (otherwise they serialize at the boundary)
            for n in range(NCHUNK):
                emit_proj(n)
                if n >= 1:
                    emit_attention(n - 1)
            flush_pending()
            emit_attention(NCHUNK - 1)

            # ---- tail: o_proj for the final chunk. The attention mm ring
            # (3 PSUM banks) is idle now; alternating out_ps between it and
            # the out ring gives a 5-deep ring so the PE never waits on the
            # scalar/vector PSUM drains.
            flush_pending()
            for k in range(16):
                pool, tag = ((mmp, "mm"), (opp, "out"))[k % 2]
                make_chain(NCHUNK - 1, k, pool, tag, fine_dma=(k >= 14))()
            flush_pending()

    return nc


def _legalize_waits(j):
    """Split multi-wait instructions: the TPB ISA gives each instruction (and
    each dynamic-DMA descriptor) a single semaphore-wait slot, and this walrus
    build errors on extras instead of splitting them. Hoist all but one wait
    into standalone EventSemaphore instructions on the issuing engine, placed
    immediately before the instruction (engine streams execute in program
    order, so the waits complete before the op issues / the descriptor posts).
    """
    n_new = 0
    for fn in j["functions"]:
        for bb in fn["blocks"]:
            insts = bb.get("instructions", [])
            out = []
            for inst in insts:
                si = inst.get("sync_info") or {}
                waits = si.get("on_wait") or []
                if len(waits) > 1:
                    for w in waits[:-1]:
                        n_new += 1
                        out.append(
                            {
                                "name": f"{inst['name']}-lw{n_new}",
                                "opcode": "EventSemaphore",
                                "engine": inst["engine"],
                                "ins": [],
                                "outs": [],
                                "debug": inst.get("debug"),
                                "sync_info": {"on_update": [], "on_wait": [w]},
                            }
                        )
                    si = dict(si)
                    si["on_wait"] = [waits[-1]]
                    inst = dict(inst)
                    inst["sync_info"] = si
                out.append(inst)
            bb["instructions"] = out
    return j


def _patch_json(nc):
    import json

    orig = nc.to_json_bytes

    def patched():
        j = json.loads(orig())
        return json.dumps(_legalize_waits(j)).encode()

    nc.to_json_bytes = patched
    return nc


_NC_CACHE = None


def _get_nc():
    global _NC_CACHE
    if _NC_CACHE is None:
        _NC_CACHE = _patch_json(build_kernel())
    return _NC_CACHE


def _prep_in_maps(hidden_states, W_qkv, W_o):
    hid = np.asarray(hidden_states, dtype=np.float32).reshape(S, D)
    # packed [chunk, kp, p, j, 512] (kb = 2*kp+j pair-interleaved so a pair
    # transfer writes each partition as one contiguous 2KB run):
    # hidT[(2*kp+j)*128+p, n*512+c]
    hidT = np.ascontiguousarray(
        hid.T.reshape(NKB // 2, 2, 128, NCHUNK, 512).transpose(3, 0, 2, 1, 4)
    ).astype(np_bf16)
    W_qkv = np.asarray(W_qkv, dtype=np.float32)
    W_o = np.asarray(W_o, dtype=np.float32)

    inv = 1.0 / (THETA ** (np.arange(0, DH, 2, dtype=np.float64) / DH))
    fr = np.arange(S, dtype=np.float64)[:, None] * inv[None, :]  # [S, 64]
    cosT = np.cos(fr).T
    sinT = np.sin(fr).T
    cos2 = np.ascontiguousarray(np.concatenate([cosT, cosT], 0)).astype(np_bf16)
    sinS = np.ascontiguousarray(np.concatenate([-sinT, sinT], 0)).astype(np_bf16)

    in_maps = []
    for i in range(NCORES):
        q_cols = W_qkv[:, 512 * i:512 * i + 512]
        k_cols = W_qkv[:, H * DH + 128 * i:H * DH + 128 * i + 128]
        v_cols = W_qkv[:, (H + HKV) * DH + 128 * i:(H + HKV) * DH + 128 * i + 128]
        wkv_i = np.ascontiguousarray(
            np.concatenate([k_cols, v_cols], axis=1)
        ).astype(np_bf16)
        wq_i = np.ascontiguousarray(q_cols).astype(np_bf16)
        wo_i = np.ascontiguousarray(W_o[512 * i:512 * i + 512, :]).astype(np_bf16)
        in_maps.append(
            {
                "hidT": hidT,
                "wkv": wkv_i,
                "wq": wq_i,
                "wo": wo_i,
                "cos2": cos2,
                "sinS": sinS,
            }
        )
    return in_maps


def _run(in_maps, trace=False, **kw):
    nc = _get_nc()
    return run_bass_kernel_spmd(
        nc, in_maps, core_ids=list(range(NCORES)), trace=trace, **kw
    )


def _gather(res):
    total = np.zeros((S, D), dtype=np.float32)
    for i in range(NCORES):
        part = np.asarray(res.results[i]["out"], dtype=np.float32)
        # unpack [qb, dc, 128, 512] -> [S, D]
        total += part.transpose(0, 2, 1, 3).reshape(S, D)
    return total.reshape(S, B, D).astype(np.float32)


def kernel(hidden_states, sequence_mask, W_qkv, W_o):
    in_maps = _prep_in_maps(hidden_states, W_qkv, W_o)
    return _gather(_run(in_maps))



# revision 25
# speedup vs baseline: 1.1979x; 1.0035x over previous
"""Trainium2 Bass kernel for causal GQA self-attention (S=2048, D=4096, H=32,
HKV=8, DH=128), tensor-parallel over 8 NeuronCores.

Sharding: head-parallel TP. Core i owns q-heads [4i..4i+4) and kv-head i:
  - qkv_proj column shard -> q [S,512], k [S,128], v [S,128]
  - RoPE + causal attention for its 4 heads (GQA group shares the kv head)
  - o_proj row shard (rows [512i..512i+512)) -> bf16 partial [S, D]
Host sums the 8 partials (the "all-reduce") and reshapes to [S, 1, D].

v2 design ("S^T-direct"): attention scores are computed directly in kv-major
layout, sT[kv, q] = K·Q^T, using the dh-major K/Q slabs that the projection
already produces — this removes all 544 PE transposes of P and the 544
PSUM->SBUF copies that made the v1 pipeline DVE-bound and let HAM oscillate.
The softmax denominator l[q] = sum_kv exp(s) is computed on the tensor engine
with an all-ones [128,128] stationary operand: same N-stream cost as any MM,
and the result lands replicated on all 128 partitions, so the per-q reciprocal
can be applied to ctx^T with a single DVE multiply (no partition broadcast).

Per-core phases, interleaved per 512-row sequence chunk n (PE stays dense):
  1. qkv projection, 6 slabs of 128 cols (4q + k + v), N=512 streams;
     q/k slabs RoPE'd in place, v slab PE-transposed to seq-major tiles.
  2. attention for q-chunk c=n: per kv-tile t: sT MM (causal suffix only) ->
     diag mask add (DVE) -> exp (scalar, direct to SBUF bf16) -> l MM + PV MM,
     software-pipelined by 2 tiles so the PE never waits on the scalar exp.
  3. o_proj row shard for chunk c=n, N=512 streams, bf16 partial out.

All matmuls run in bf16 with fp32 PSUM accumulation. Softmax runs without
max-subtraction (logits are O(10) here, far inside fp32 exp range).
"""

import sys

sys.path.insert(0, "/opt/trn_rl_repo")

import numpy as np
import ml_dtypes
from contextlib import ExitStack

import concourse.bass as bass
import concourse.tile as tile
from concourse import mybir
from concourse.bass_utils import run_bass_kernel_spmd
from concourse.masks import make_identity

S, B, D = 2048, 1, 4096
H, HKV, DH = 32, 8, 128
NCORES = 8
HQ = H // HKV  # q heads per core = 4
NSLAB = HQ + 2  # 4 q slabs + k + v
THETA = 10000.0
SCALE = 1.0 / float(np.sqrt(DH))

BF16 = mybir.dt.bfloat16
F32 = mybir.dt.float32
np_bf16 = ml_dtypes.bfloat16

NKB = D // 128  # 32 contraction blocks for the projections
NQB = S // 128  # 16 seq blocks of 128
NCHUNK = S // 512  # 4 sequence chunks of 512


def build_kernel() -> bass.Bass:
    nc = bass.Bass()

    # hidT packed host-side as contiguous [chunk, kb, 128, 512] tiles so each
    # DMA is one dense 128KB block (strided 1KB rows run at ~half DMA rate)
    hidT_e = nc.declare_dram_parameter(
        "hidT", [NCHUNK, NKB // 2, 128, 2, 512], BF16, isOutput=False
    )
    # wqkv split into two contiguous tensors, cols [k|v] and [q0..q3], so the
    # ramp can deliver the k+v columns of a kb block (64KB) independently of
    # its q columns: the first matmul starts ~2.5us earlier and each kb
    # unlocks in 3 sub-deliveries instead of one 320KB step
    wkv_e = nc.declare_dram_parameter("wkv", [D, 2 * DH], BF16, isOutput=False)
    wq_e = nc.declare_dram_parameter("wq", [D, HQ * DH], BF16, isOutput=False)
    wo_e = nc.declare_dram_parameter("wo", [HQ * DH, D], BF16, isOutput=False)
    # cos2 = [cos; cos], sinS = [-sin; sin]  (dh-major halves stacked)
    cos_e = nc.declare_dram_parameter("cos2", [128, S], BF16, isOutput=False)
    sin_e = nc.declare_dram_parameter("sinS", [128, S], BF16, isOutput=False)
    # out packed [qb, dc, 128, 512]; host unpacks to [S, D]
    out_e = nc.declare_dram_parameter("out", [NQB, 8, 128, 512], BF16, isOutput=True)

    hidT = hidT_e[:]
    wkv = wkv_e[:]
    wq = wq_e[:]
    wo = wo_e[:]
    out = out_e[:]

    with tile.TileContext(nc) as tc, ExitStack() as ctx:
        singles = ctx.enter_context(tc.tile_pool(name="singles", bufs=1))

        # ---- persistent SBUF state ----
        wqkv_sb = singles.tile([128, NKB, NSLAB * DH], BF16)
        wo_sb = singles.tile([128, HQ, D], BF16)
        cos_sb = singles.tile([128, S], BF16)
        sin_sb = singles.tile([128, S], BF16)
        ident = singles.tile([128, 128], BF16)
        ones_sb = singles.tile([128, 128], BF16)
        maskT = singles.tile([128, 128], F32)
        # q slabs (m=0..3) + k slab (m=4), dh-major [dh, S], RoPE'd
        qkT_sb = singles.tile([128, HQ + 1, S], BF16)
        # V seq-major: tile t = rows [128t..128t+128) x [dh 128]
        v_sb = singles.tile([128, NQB, DH], BF16)
        # ctx^T per q-head slab [dh, S], softmax-normalized
        ctxT_sb = singles.tile([128, HQ, S], BF16)

        make_identity(nc, ident)
        nc.vector.memset(ones_sb, 1.0)
        # maskT[kv, q] = 0 where q >= kv (valid, diag incl), else -1e9
        nc.gpsimd.memset(maskT, 0.0)
        nc.gpsimd.affine_select(
            out=maskT,
            in_=maskT,
            compare_op=mybir.AluOpType.is_ge,
            fill=-1e9,
            base=0,
            pattern=[[1, 128]],
            channel_multiplier=-1,
        )

        with (
            tc.tile_pool(name="hidp", bufs=22) as hidp,
            tc.tile_pool(name="ropep", bufs=4) as ropep,
            tc.tile_pool(name="vtmp", bufs=2) as vtmpp,
            tc.tile_pool(name="ptp", bufs=4) as ptp,
            tc.tile_pool(name="paccp", bufs=2) as paccp,
            tc.tile_pool(name="linvp", bufs=2) as linvp,
            tc.tile_pool(name="outsb", bufs=4) as osp,
            # PSUM budget: 3 (mm: proj slabs + sT) + 1 (l) + 2 (ctx) + 2
            # (out: o_proj + v-transpose) = 8 banks
            tc.tile_pool(name="ps_mm", bufs=3, space="PSUM") as mmp,
            tc.tile_pool(name="ps_l", bufs=1, space="PSUM") as lpp,
            tc.tile_pool(name="ps_ctx", bufs=2, space="PSUM") as cpp,
            tc.tile_pool(name="ps_out", bufs=2, space="PSUM") as opp,
        ):
            # Deferred per-head softmax normalize: the final l matmul + the
            # Ln/Exp(-x) reciprocal + ctx multiply are emitted under PE cover
            # of the NEXT head's first score matmuls (or the next chunk's
            # first projection matmuls), so the exp->l-accumulate latency
            # never stalls the tensor engine.
            pending = [None]

            def flush_pending():
                if pending[0] is None:
                    return
                cc, hh, l_ps_, ctx_ps_, pacc_ = pending[0]
                pending[0] = None
                if pacc_ is not None:
                    nc.tensor.matmul(l_ps_, ones_sb, pacc_, start=True, stop=True)
                lnl = linvp.tile([128, 512], F32, name=f"lnl_{cc}_{hh}", tag="lnl")
                nc.scalar.activation(lnl, l_ps_, mybir.ActivationFunctionType.Ln)
                linv = linvp.tile(
                    [128, 512], F32, name=f"linv_{cc}_{hh}", tag="linv"
                )
                nc.scalar.activation(
                    linv, lnl, mybir.ActivationFunctionType.Exp, scale=-1.0
                )
                nc.vector.tensor_mul(
                    ctxT_sb[:, hh, cc * 512:(cc + 1) * 512], ctx_ps_, linv
                )

            def issue_ht(n):
                # paired DMAs (2 kb-blocks per transfer): the sync sequencer
                # dispatches each dma_start serially at ~0.6us, so transfer
                # count, not just bytes, gates chunk-0 delivery
                htp = []
                for kp in range(NKB // 2):
                    # first few transfers split into kb singles (interleaved
                    # kv-cols / ht / q-cols so the very first matmul's inputs
                    # land first); afterwards kb-pairs win on dispatch count
                    split = n == 0 and kp < 3
                    t_ = hidp.tile(
                        [128, 2, 512], BF16, name=f"ht_{n}_{kp}", tag="ht"
                    )
                    if n == 0:
                        if split:
                            for j in range(2):
                                kb = 2 * kp + j
                                nc.sync.dma_start(
                                    out=wqkv_sb[:, kb, 0:256],
                                    in_=wkv[kb * 128:(kb + 1) * 128, :],
                                )
                                nc.sync.dma_start(
                                    out=t_[:, j, :], in_=hidT[n, kp, :, j, :]
                                )
                                nc.sync.dma_start(
                                    out=wqkv_sb[:, kb, 256:768],
                                    in_=wq[kb * 128:(kb + 1) * 128, :],
                                )
                        else:
                            nc.sync.dma_start(
                                out=wqkv_sb[:, 2 * kp:2 * kp + 2, 0:256],
                                in_=wkv[kp * 256:(kp + 1) * 256, :].rearrange(
                                    "(j p) c -> p j c", p=128
                                ),
                            )
                            nc.sync.dma_start(
                                out=t_,
                                in_=hidT[n, kp],
                            )
                            nc.sync.dma_start(
                                out=wqkv_sb[:, 2 * kp:2 * kp + 2, 256:768],
                                in_=wq[kp * 256:(kp + 1) * 256, :].rearrange(
                                    "(j p) c -> p j c", p=128
                                ),
                            )
                        if kp == 2:
                            nc.sync.dma_start(out=cos_sb, in_=cos_e[:])
                            nc.sync.dma_start(out=sin_sb, in_=sin_e[:])
                    else:
                        nc.sync.dma_start(
                            out=t_,
                            in_=hidT[n, kp],
                        )
                    htp.append(t_)
                return htp

            htp_map = {0: issue_ht(0)}

            def make_chain(oc, dp_idx, ps_pool, ps_tag, fine_dma=False):
                iq, dp = dp_idx // 4, dp_idx % 4

                def chain():
                    qb = 4 * oc + iq
                    out_sb = osp.tile(
                        [128, 2, 512], BF16, name="out_sb", tag="out_sb"
                    )
                    for half in range(2):
                        dc = 2 * dp + half
                        out_ps = ps_pool.tile(
                            [128, 512], F32, name=f"out_ps_{qb}_{dc}",
                            tag=ps_tag,
                        )
                        for h in range(HQ):
                            nc.tensor.matmul(
                                out_ps,
                                ctxT_sb[:, h, qb * 128:(qb + 1) * 128],
                                wo_sb[:, h, dc * 512:(dc + 1) * 512],
                                start=(h == 0),
                                stop=(h == HQ - 1),
                            )
                        if half == 0:
                            nc.scalar.copy(out_sb[:, 0, :], out_ps)
                        else:
                            nc.vector.tensor_copy(out_sb[:, 1, :], out_ps)
                    if fine_dma:
                        for half in range(2):
                            nc.sync.dma_start(
                                out=out[qb, 2 * dp + half],
                                in_=out_sb[:, half, :],
                            )
                    else:
                        nc.sync.dma_start(
                            out=out[qb, 2 * dp:2 * dp + 2].rearrange(
                                "a p c -> p a c"
                            ),
                            in_=out_sb,
                        )

                return chain

            vts = {}
            deferred_fin = []

            def emit_proj(n):
                # ---- qkv projection for seq chunk n ----
                ht = [htp_map[n][kb // 2][:, kb % 2, :] for kb in range(NKB)]
                sl = slice(n * 512, (n + 1) * 512)
                # k slab first so its RoPE is long done when attention starts;
                # v 5th so its staging copy beats the h0 transposes;
                # (slab_idx in qkT_sb/v, column offset in wqkv)
                # All 6 slabs run kb-synchronously: hidp ring slots then
                # free progressively through the phase (slab-sequential order
                # frees them only during the last slab's sweep, compressing
                # the next chunk's hidT delivery into the phase tail), and at
                # chunk 0 the PE work unlocked per arriving (wkv, ht, wq)
                # kb-pair matches the DMA ramp's delivery rate.
                # Accumulator rings, chosen by when their previous occupant
                # frees: k first (ctx ring slot freed in the prior attention),
                # v/q0/q1 on the mm ring (sT slots), q3 on the l ring (freed
                # by the pending flush's early Ln), q2 last on the ctx ring
                # (freed by that flush's final DVE multiply, ~1.6us in).
                # Emission order also matches the ramp delivery order
                # kv-cols -> ht -> q-cols of each kb block.
                # order: the mm-ring slabs (v,q0,q1) finish first (their
                # PSUM slots are reused by the next attention's first sT
                # tiles within ~1us), then k (ctx slot, reused at ~+1.3us),
                # then q3/q2 whose borrowed l/ctx slots aren't touched for
                # 4-8us. The v slab leads so its staging copy is long done
                # before the attention-h0 transposes.
                SLABS = ((5, 128), (0, 256), (1, 384), (4, 0), (3, 640), (2, 512))
                groups = [SLABS]
                slab_pools = [mmp, mmp, mmp, cpp, lpp, cpp]
                slab_tags = ["mm", "mm", "mm", "ctx", "l", "ctx"]
                vt_box = [None]

                def finish_slab(m, ps, fn=None):
                    fsl = sl if fn is None else slice(fn * 512, (fn + 1) * 512)
                    if m < NSLAB - 1:
                        # q or k slab: copy out of PSUM, then RoPE in place
                        slab = qkT_sb[:, m, fsl]
                        nc.scalar.copy(slab, ps)
                        rot = ropep.tile([128, 512], BF16, name="rot", tag="rot")
                        nc.sync.dma_start(out=rot[0:64, :], in_=qkT_sb[64:128, m, fsl])
                        nc.sync.dma_start(out=rot[64:128, :], in_=qkT_sb[0:64, m, fsl])
                        rt = ropep.tile([128, 512], BF16, name="rt", tag="rt")
                        nc.vector.tensor_mul(rt, rot, sin_sb[:, fsl])
                        nc.vector.tensor_mul(slab, slab, cos_sb[:, fsl])
                        nc.vector.tensor_add(slab, slab, rt)
                    else:
                        # v slab: stage to SBUF; PE-transposed to seq-major
                        # inside the attention A-loop (keeps PE dense)
                        vt_box[0] = vtmpp.tile([128, 512], BF16, name="vt", tag="vt")
                        nc.scalar.copy(vt_box[0], ps)

                grp = groups[0]
                pss = []
                for si, (m, _) in enumerate(grp):
                    pss.append(
                        slab_pools[si].tile(
                            [128, 512], F32, name=f"proj_ps_{n}_{m}",
                            tag=slab_tags[si],
                        )
                    )
                # last head's softmax normalize of the chunk-before-last.
                # Must precede the kb loop: q2's first matmul waits on the
                # ctx-ring slot this flush's DVE multiply frees, and that
                # multiply's l matmul must come first in the PE queue.
                flush_pending()
                # Each slab's sweep lags the previous by 2 kb: the six PSUM
                # drains then fire ~2.5us apart, each overlapped by the later
                # slabs' remaining matmuls, instead of bunching at phase end
                # in front of the next attention's exps in the scalar queue.
                # It also gives the ramp's q-piece deliveries (chunk 0) 4+
                # kb of slack behind their kv/ht pieces, and the borrowed
                # l/ctx-ring slots (q3/q2) time to clear the pending flush.
                LAGS = (0, 2, 4, 6, 8, 10)
                for step in range(NKB + LAGS[-1] + 1):
                    for gi, (m, coff) in enumerate(grp):
                        kb_g = step - LAGS[gi]
                        if not (0 <= kb_g < NKB):
                            continue
                        nc.tensor.matmul(
                            pss[gi],
                            wqkv_sb[:, kb_g, coff:coff + 128],
                            ht[kb_g],
                            start=(kb_g == 0),
                            stop=(kb_g == NKB - 1),
                        )
                        if kb_g == NKB - 1:
                            if n >= 1 and gi >= 4:
                                # q3/q2 finish at phase end; their scalar
                                # copies would queue in front of the next
                                # attention's first exps (its critical path),
                                # and their PSUM slots/RoPE results are not
                                # needed for another 4-8us -- defer them into
                                # that attention's head 0
                                deferred_fin.append(
                                    (lambda mm, pp, nn: lambda: finish_slab(
                                        mm, pp, fn=nn
                                    ))(m, pss[gi], n)
                                )
                            else:
                                # slab done: drain + RoPE it now, under cover
                                # of the remaining slabs' matmuls
                                finish_slab(m, pss[gi])
                    if step == NKB - 1 and n + 1 < NCHUNK:
                        # prefetch next chunk's hidden states (most finishes
                        # already emitted; their rot DMAs never head-of-line-
                        # block these dispatches for long)
                        htp_map[n + 1] = issue_ht(n + 1)
                vts[n] = vt_box[0]
                if n == 0:
                    # o_proj weights: first needed by the chains in attn c1
                    for h in range(HQ):
                        nc.sync.dma_start(
                            out=wo_sb[:, h, :], in_=wo[h * 128:(h + 1) * 128, :]
                        )

            def emit_attention(c):
                # ---- attention for q-chunk c, with o_proj chains for chunk
                # c-1 interleaved between score tiles. The attention inner
                # loop is scalar-bound (exp of a [128,512] tile ~530ns vs
                # ~432ns of PE per tile), so without filler the PE idles
                # ~100ns/tile waiting on exp; each interleaved chain adds
                # ~1.7us of exp-independent PE work.
                vt = vts.pop(c)
                chains = []
                if c >= 1:
                    chains = [
                        make_chain(c - 1, k, opp, "out") for k in range(16)
                    ]
                ci = [0]

                def emit_chain():
                    if ci[0] < len(chains):
                        chains[ci[0]]()
                        ci[0] += 1

                ntile = 4 * (c + 1)
                slots = HQ * ntile
                stride = max(1, slots // 16)
                slot = [0]
                for h in range(HQ):
                    # (t, qoff, w): kv tile t covers seq [128t, 128t+128); for
                    # diagonal tiles only q >= 128t attends -> stream suffix
                    tiles = []
                    for t in range(ntile):
                        qoff = max(0, 128 * (t - 4 * c))
                        tiles.append((t, qoff, 512 - qoff))
                    nt = len(tiles)
                    l_ps = lpp.tile([128, 512], F32, name=f"l_ps_{c}_{h}", tag="l")
                    ctx_ps = cpp.tile(
                        [128, 512], F32, name=f"ctx_ps_{c}_{h}", tag="ctx"
                    )
                    # c>=1: sum pT tiles on DVE; a single ones-matmul in the
                    # flush then reduces over kv (one PE stream instead of one
                    # per tile). c=0 keeps the per-tile ones-matmuls: A0 has
                    # no o_proj chains to fill the exp-bound stretches, and
                    # the l matmuls (which run after each exp) are free PE
                    # fill there.
                    if c >= 1:
                        pacc = paccp.tile(
                            [128, 512], BF16, name=f"pacc_{c}_{h}", tag="pacc"
                        )
                        nc.vector.memset(pacc, 0.0)
                    else:
                        pacc = None
                    pT_tiles = {}

                    def stage_a(i):
                        t, qoff, w = tiles[i]
                        sT = mmp.tile(
                            [128, 512], F32, name=f"sT_{c}_{h}_{t}", tag="mm"
                        )
                        nc.tensor.matmul(
                            sT[:, 0:w],
                            qkT_sb[:, HQ, t * 128:(t + 1) * 128],
                            qkT_sb[:, h, c * 512 + qoff:(c + 1) * 512],
                            start=True,
                            stop=True,
                        )
                        if t >= 4 * c:
                            # diagonal tile: first 128 streamed q's need mask
                            nc.vector.tensor_add(sT[:, 0:128], sT[:, 0:128], maskT)
                        pT = ptp.tile([128, 512], BF16, name=f"pT_{c}_{h}_{t}", tag="pt")
                        nc.scalar.activation(
                            pT[:, 0:w],
                            sT[:, 0:w],
                            mybir.ActivationFunctionType.Exp,
                            scale=SCALE,
                        )
                        pT_tiles[i] = pT
                        if pacc is not None:
                            # accumulate sum-over-tiles on DVE; the single
                            # all-ones matmul in the flush reduces over kv
                            nc.vector.tensor_add(
                                pacc[:, qoff:512], pacc[:, qoff:512], pT[:, 0:w]
                            )

                    def stage_c(i):
                        t, qoff, w = tiles[i]
                        pT = pT_tiles.pop(i)
                        first, last = i == 0, i == nt - 1
                        if pacc is None:
                            nc.tensor.matmul(
                                l_ps[:, qoff:512],
                                ones_sb,
                                pT[:, 0:w],
                                start=first,
                                stop=last,
                            )
                        nc.tensor.matmul(
                            ctx_ps[:, qoff:512],
                            v_sb[:, t, :],
                            pT[:, 0:w],
                            start=first,
                            stop=last,
                        )

                    DEPTH = 2
                    # transposes early in h0: they are exp-independent PE
                    # fill right in the attention-start window where the
                    # scalar engine is still draining proj copies + first
                    # exps (v_sb results are not needed before stage_c(4c))
                    vt0 = 0
                    for i in range(nt):
                        stage_a(i)
                        if h == 0 and vt0 <= i < vt0 + 4:
                            # chunk c's v tiles: dh-major -> seq-major, first
                            # needed by stage_c(4c); scheduled 4 tiles early
                            # so the DVE copies run mid-head, clear of the
                            # head-start rope/chain burst on DVE
                            j = i - vt0
                            vtp = opp.tile([128, 128], BF16, name="vtp", tag="out")
                            nc.tensor.transpose(
                                vtp, vt[:, j * 128:(j + 1) * 128], ident
                            )
                            nc.vector.tensor_copy(v_sb[:, 4 * c + j, :], vtp)
                        if i == 1:
                            flush_pending()
                        if h == 0 and i in (2, 3) and deferred_fin:
                            # drain a deferred proj slab now: its scalar copy
                            # queues behind this head's first exps
                            deferred_fin.pop(0)()
                        if i >= DEPTH:
                            stage_c(i - DEPTH)
                        slot[0] += 1
                        if slot[0] % stride == 1 or stride == 1:
                            emit_chain()
                    for i in range(max(0, nt - DEPTH), nt):
                        stage_c(i)
                    pending[0] = (c, h, l_ps, ctx_ps, pacc)
                while ci[0] < len(chains):
                    emit_chain()

            # phase order P0 P1 A0 P2 A1 P3 A2 A3 tail: attention c runs
            # after proj c+1, whose matmuls cover chunk c's PSUM drains and
            # RoPE chains (otherwise they serialize at the boundary)
            for n in range(NCHUNK):
                emit_proj(n)
                if n >= 1:
                    emit_attention(n - 1)
            flush_pending()
            emit_attention(NCHUNK - 1)

            # ---- tail: o_proj for the final chunk. The attention mm ring
            # (3 PSUM banks) is idle now; alternating out_ps between it and
            # the out ring gives a 5-deep ring so the PE never waits on the
            # scalar/vector PSUM drains.
            flush_pending()
            for k in range(16):
                pool, tag = ((mmp, "mm"), (opp, "out"))[k % 2]
                make_chain(NCHUNK - 1, k, pool, tag, fine_dma=(k >= 14))()
            flush_pending()

    return nc


def _legalize_waits(j):
    """Split multi-wait instructions: the TPB ISA gives each instruction (and
    each dynamic-DMA descriptor) a single semaphore-wait slot, and this walrus
    build errors on extras instead of splitting them. Hoist all but one wait
    into standalone EventSemaphore instructions on the issuing engine, placed
    immediately before the instruction (engine streams execute in program
    order, so the waits complete before the op issues / the descriptor posts).
    """
    n_new = 0
    for fn in j["functions"]:
        for bb in fn["blocks"]:
            insts = bb.get("instructions", [])
            out = []
            for inst in insts:
                si = inst.get("sync_info") or {}
                waits = si.get("on_wait") or []
                if len(waits) > 1:
                    for w in waits[:-1]:
                        n_new += 1
                        out.append(
                            {
                                "name": f"{inst['name']}-lw{n_new}",
                                "opcode": "EventSemaphore",
                                "engine": inst["engine"],
                                "ins": [],
                                "outs": [],
                                "debug": inst.get("debug"),
                                "sync_info": {"on_update": [], "on_wait": [w]},
                            }
                        )
                    si = dict(si)
                    si["on_wait"] = [waits[-1]]
                    inst = dict(inst)
                    inst["sync_info"] = si
                out.append(inst)
            bb["instructions"] = out
    return j


def _patch_json(nc):
    import json

    orig = nc.to_json_bytes

    def patched():
        j = json.loads(orig())
        return json.dumps(_legalize_waits(j)).encode()

    nc.to_json_bytes = patched
    return nc


_NC_CACHE = None


def _get_nc():
    global _NC_CACHE
    if _NC_CACHE is None:
        _NC_CACHE = _patch_json(build_kernel())
    return _NC_CACHE


def _prep_in_maps(hidden_states, W_qkv, W_o):
    hid = np.asarray(hidden_states, dtype=np.float32).reshape(S, D)
    # packed [chunk, kp, p, j, 512] (kb = 2*kp+j pair-interleaved so a pair
    # transfer writes each partition as one contiguous 2KB run):
    # hidT[(2*kp+j)*128+p, n*512+c]
    hidT = np.ascontiguousarray(
        hid.T.reshape(NKB // 2, 2, 128, NCHUNK, 512).transpose(3, 0, 2, 1, 4)
    ).astype(np_bf16)
    W_qkv = np.asarray(W_qkv, dtype=np.float32)
    W_o = np.asarray(W_o, dtype=np.float32)

    inv = 1.0 / (THETA ** (np.arange(0, DH, 2, dtype=np.float64) / DH))
    fr = np.arange(S, dtype=np.float64)[:, None] * inv[None, :]  # [S, 64]
    cosT = np.cos(fr).T
    sinT = np.sin(fr).T
    cos2 = np.ascontiguousarray(np.concatenate([cosT, cosT], 0)).astype(np_bf16)
    sinS = np.ascontiguousarray(np.concatenate([-sinT, sinT], 0)).astype(np_bf16)

    in_maps = []
    for i in range(NCORES):
        q_cols = W_qkv[:, 512 * i:512 * i + 512]
        k_cols = W_qkv[:, H * DH + 128 * i:H * DH + 128 * i + 128]
        v_cols = W_qkv[:, (H + HKV) * DH + 128 * i:(H + HKV) * DH + 128 * i + 128]
        wkv_i = np.ascontiguousarray(
            np.concatenate([k_cols, v_cols], axis=1)
        ).astype(np_bf16)
        wq_i = np.ascontiguousarray(q_cols).astype(np_bf16)
        wo_i = np.ascontiguousarray(W_o[512 * i:512 * i + 512, :]).astype(np_bf16)
        in_maps.append(
            {
                "hidT": hidT,
                "wkv": wkv_i,
                "wq": wq_i,
                "wo": wo_i,
                "cos2": cos2,
                "sinS": sinS,
            }
        )
    return in_maps


def _run(in_maps, trace=False, **kw):
    nc = _get_nc()
    return run_bass_kernel_spmd(
        nc, in_maps, core_ids=list(range(NCORES)), trace=trace, **kw
    )


def _gather(res):
    total = np.zeros((S, D), dtype=np.float32)
    for i in range(NCORES):
        part = np.asarray(res.results[i]["out"], dtype=np.float32)
        # unpack [qb, dc, 128, 512] -> [S, D]
        total += part.transpose(0, 2, 1, 3).reshape(S, D)
    return total.reshape(S, B, D).astype(np.float32)


def kernel(hidden_states, sequence_mask, W_qkv, W_o):
    in_maps = _prep_in_maps(hidden_states, W_qkv, W_o)
    return _gather(_run(in_maps))

